# revision 25
# baseline (speedup 1.0000x reference)
"""Trainium2 Bass kernel for nn_AE_89343909691943 (multi-view AE loss_fn).

8-core SPMD strategy (data-parallel over the N=4096 sample axis, 512 rows/core):
  * 6 per-view encoder MLPs run row-sharded in transposed-activation layout
    (features on partitions), so bias+ReLU fuse into PSUM evacuation and
    BatchNorm stats are per-partition reductions.
  * BN batch stats via a tiny AllReduce; fused z via an AllGather of z.
  * The N x N similarity matrices are never materialized.  The loss
      loss_i = mean((w (sims_i - sim2) w)^2)
    is computed exactly as
      (||X^T D_{u^2} X||_F^2 - 2||X^T D_{uv} Z||_F^2 + ||Z^T D_{v^2} Z||_F^2)/N^2
    with u = w/||x_row||, v = w/||z_row||, which shrinks the gram work from
    O(N^2 d) to O(N d^2) and keeps everything on the TensorEngine.
  * All GEMMs run in bf16 with fp32 PSUM accumulation (validated ~0.5% worst
    output error, loss error ~1e-5).
  * DMAs are batched (one instruction per weight m-panel / k-group) to keep
    the Sync sequencer's descriptor generation off the critical path.
"""

import sys

for _p in ("/opt/trn_rl_repo", "/root/.axon_site/_ro/trn_rl_repo"):
    if _p not in sys.path:
        sys.path.insert(0, _p)

import math

import ml_dtypes
import numpy as np

import concourse.bass as bass
import concourse.mybir as mybir
import concourse.tile as tile
from concourse import bacc
from concourse.bass_utils import run_bass_kernel_spmd
from concourse.masks import make_identity

BF16 = mybir.dt.bfloat16
F32 = mybir.dt.float32
AF = mybir.ActivationFunctionType
ALU = mybir.AluOpType

N = 4096
NCORES = 8
R = N // NCORES  # 512 rows per core
NZ = 128
NL = 80
NV = 6
D = [1024, 1536, 2048, 1280, 896, 1024]
D0 = [819, 1229, 1638, 1024, 717, 819]
D0P = [896, 1280, 1664, 1024, 768, 896]
HID = 1500
HIDP = 1536
BN_EPS = 1e-5
KB = N // 128  # 32 row-blocks of 128
RB = R // 128  # 4 local row-blocks
KGRP = 4  # gram k-blocks fetched per DMA

NASLOT = 4  # gram-A pieces per core (uniform across cores)
NBSLOT = 2  # gram-B (cross term) units per core


def _a_pieces():
    """(view, chunk_start, chunk_width, m_start) gram-A pieces.

    A_i = (D_u X_i)^T (D_u X_i) is computed in 512-wide column chunks; for
    each chunk only m <= chunk_end 512-wide lhsT pieces are needed (symmetric
    matrix; strictly-below pieces get a sqrt(2) factor folded into the host
    scaling so ||.||^2 counts them twice)."""
    pieces = []
    for v in range(NV):
        for cs in range(0, D[v], 512):
            w = min(512, D[v] - cs)
            for ms in range(0, cs + 1, 512):
                pieces.append((v, cs, w, ms))
    return pieces


def _b_units():
    units = []
    for v in range(NV):
        for cs in range(0, D[v], 512):
            units.append((v, cs, min(512, D[v] - cs)))
    return units


A_PIECES = _a_pieces()  # 31 pieces -> 32 slots (one zero slot)
B_UNITS = _b_units()  # 16 units -> exactly 2 per core
assert len(A_PIECES) <= NASLOT * NCORES
assert len(B_UNITS) == NBSLOT * NCORES


def build_nc(stages=frozenset({"enc", "ar", "a", "ag", "bc"})):
    nc = bacc.Bacc("TRN2", target_bir_lowering=False, debug=False,
                   num_devices=NCORES)

    ENC = "enc" in stages
    AR = "ar" in stages
    GA = "a" in stages
    AG = "ag" in stages
    BC = "bc" in stages

    # ---------------- parameters ----------------
    # weights are host-swizzled to [m_tile, 128, K*128]:
    #   w[m, p, k*128+f] = W_padded[k*128 + p, m*128 + f]
    xt = [nc.declare_dram_parameter(f"xt{v}", [D[v], R], BF16, isOutput=False)
          for v in range(NV)]
    w1 = [nc.declare_dram_parameter(
        f"w1_{v}", [D0P[v] // 128, 128, D[v]], BF16, isOutput=False)
        for v in range(NV)]
    w2 = [nc.declare_dram_parameter(
        f"w2_{v}", [HIDP // 128, 128, D0P[v]], BF16, isOutput=False)
        for v in range(NV)]
    wz = [nc.declare_dram_parameter(
        f"wz_{v}", [1, 128, HIDP], BF16, isOutput=False)
        for v in range(NV)]
    # all small per-partition constants packed into one [128, CCOLS] param:
    # cols: per view [b1c(D0P/128) b2c(12) bz gam bet] then bw(2*32) cw(32)
    # regb(1) eps(1)
    CCOLS = sum(D0P[v] // 128 + HIDP // 128 + 3 for v in range(NV)) + 3 * KB + 2
    consts = nc.declare_dram_parameter("consts", [128, CCOLS], F32,
                                       isOutput=False)
    regw = nc.declare_dram_parameter("regw", [NZ, NL], BF16, isOutput=False)
    nwl = nc.declare_dram_parameter("nwl", [NV, R], F32, isOutput=False)
    ga_l = nc.declare_dram_parameter("ga_l", [NASLOT, N, 512], BF16, isOutput=False)
    ga_r = nc.declare_dram_parameter("ga_r", [NASLOT, N, 512], BF16, isOutput=False)
    gb_r = nc.declare_dram_parameter("gb_r", [NBSLOT, N, 512], BF16, isOutput=False)

    yo = nc.declare_dram_parameter("yo", [R, NL], F32, isOutput=True)
    zo = nc.declare_dram_parameter("zo", [R, NZ], F32, isOutput=True)
    zso = nc.declare_dram_parameter("zso", [NV, R, NZ], F32, isOutput=True)
    yspo = nc.declare_dram_parameter("yspo", [NV, R, NL], F32, isOutput=True)
    lossp = nc.declare_dram_parameter("lossp", [128, 1], F32, isOutput=True)

    with tile.TileContext(nc) as tc:
        with (
            tc.tile_pool(name="const", bufs=1) as const,
            tc.tile_pool(name="persist", bufs=1) as pp,
            tc.tile_pool(name="small", bufs=2) as sp,
            tc.tile_pool(name="xr", bufs=2) as xr,
            tc.tile_pool(name="h1p", bufs=14) as h1p,
            tc.tile_pool(name="h2p", bufs=13) as h2p,
            tc.tile_pool(name="wp", bufs=3) as wp,
            tc.tile_pool(name="gp", bufs=3) as gp,
            tc.tile_pool(name="gpb", bufs=5) as gpb,
            tc.tile_pool(name="zgp", bufs=4) as zgp,
            tc.tile_pool(name="zgf", bufs=1) as zgf,
            tc.tile_pool(name="ps_enc", bufs=2, space="PSUM") as ps_enc,
            tc.tile_pool(name="ps_a", bufs=4, space="PSUM") as ps_a,
            tc.tile_pool(name="ps_sm", bufs=2, space="PSUM") as ps_sm,
            tc.tile_pool(name="dram", bufs=1, space="DRAM") as dram,
        ):
            # ---------------- constants / small loads ----------------
            ident = const.tile([128, 128], F32)
            make_identity(nc, ident)

            call = const.tile([128, CCOLS], F32, tag="call")
            nc.gpsimd.dma_start(call[:], consts.ap())
            b1c, b2c, bzc, gamc, betc = [], [], [], [], []
            off = 0
            for v in range(NV):
                b1c.append(call[:, off:off + D0P[v] // 128])
                off += D0P[v] // 128
                b2c.append(call[:, off:off + HIDP // 128])
                off += HIDP // 128
                bzc.append(call[:, off:off + 1])
                gamc.append(call[:, off + 1:off + 2])
                betc.append(call[:, off + 2:off + 3])
                off += 3
            bw_t = [call[:, off + s * KB:off + (s + 1) * KB]
                    for s in range(NBSLOT)]
            off += NBSLOT * KB
            cw_t = call[:, off:off + KB]
            off += KB
            regb_t = call[:NL, off:off + 1]
            epsc = call[:, off + 1:off + 2]
            regw_t = const.tile([128, NL], BF16, tag="regw")
            nc.gpsimd.dma_start(regw_t[:], regw.ap())
            nwb_all = const.tile([128, NV, R], F32, tag="nwb")
            nc.gpsimd.dma_start(
                nwb_all[:],
                nwl.ap().rearrange("(o v) f -> o v f", o=1)
                .to_broadcast((128, NV, R)))
            nwb = [nwb_all[:, v, :] for v in range(NV)]
            stats = pp.tile([128, 2 * NV], F32, tag="stats")
            nc.vector.memset(stats[:], 0.0)
            acc = pp.tile([128, 24], F32, tag="acc")
            nc.vector.memset(acc[:], 0.0)
            trash = pp.tile([128, 512], BF16, tag="trash")

            # ---------------- encoders (transposed activations) ---------
            zlT = []
            for v in range(NV if ENC else 0):
                kd = D[v] // 128
                km = D0P[v] // 128
                kh = HIDP // 128
                xtt = xr.tile([128, 16, R], BF16, tag="xt")
                nc.scalar.dma_start(
                    xtt[:, :kd, :], xt[v].ap().rearrange("(k p) f -> p k f", p=128))
                # h1T = relu(W1^T x^T + b1)
                h1t = []
                for m in range(km):
                    wt = wp.tile([128, 2048], BF16, tag="w")
                    nc.sync.dma_start(wt[:, :D[v]], w1[v].ap()[m])
                    psum = ps_enc.tile([128, R], F32, tag="enc")
                    for k in range(kd):
                        nc.tensor.matmul(psum[:], wt[:, k * 128:(k + 1) * 128],
                                         xtt[:, k, :],
                                         start=(k == 0), stop=(k == kd - 1))
                    h = h1p.tile([128, R], BF16, tag="h1")
                    nc.scalar.activation(h[:], psum[:], AF.Relu,
                                         bias=b1c[v][:, m:m + 1])
                    h1t.append(h)
                # h2T = relu(W2^T h1T + b2)
                h2t = []
                for m in range(kh):
                    wt = wp.tile([128, 2048], BF16, tag="w")
                    nc.sync.dma_start(wt[:, :D0P[v]], w2[v].ap()[m])
                    psum = ps_enc.tile([128, R], F32, tag="enc")
                    for k in range(km):
                        nc.tensor.matmul(psum[:], wt[:, k * 128:(k + 1) * 128],
                                         h1t[k][:],
                                         start=(k == 0), stop=(k == km - 1))
                    h = h2p.tile([128, R], BF16, tag="h2")
                    nc.scalar.activation(h[:], psum[:], AF.Relu,
                                         bias=b2c[v][:, m:m + 1])
                    h2t.append(h)
                # zlT = Wz^T h2T + bz  (fp32, keep; stats fused)
                wt = wp.tile([128, 2048], BF16, tag="w")
                nc.sync.dma_start(wt[:, :HIDP], wz[v].ap()[0])
                psum = ps_enc.tile([128, R], F32, tag="enc")
                for k in range(kh):
                    nc.tensor.matmul(psum[:], wt[:, k * 128:(k + 1) * 128],
                                     h2t[k][:],
                                     start=(k == 0), stop=(k == kh - 1))
                zt = pp.tile([128, R], F32, tag=f"zl{v}")
                ssum = pp.tile([128, 1], F32, tag=f"ssum{v}", name=f"ssum{v}")
                ssq = pp.tile([128, 1], F32, tag=f"ssq{v}", name=f"ssq{v}")
                nc.scalar.activation(zt[:], psum[:], AF.Identity,
                                     bias=bzc[v], accum_out=ssum[:])
                nc.scalar.activation(trash[:], zt[:], AF.Square,
                                     accum_out=ssq[:])
                nc.vector.tensor_copy(stats[:, v:v + 1], ssum[:])
                last_stats_inst = nc.vector.tensor_copy(
                    stats[:, NV + v:NV + v + 1], ssq[:])
                zlT.append(zt)
            if not ENC:
                for v in range(NV):
                    zt = pp.tile([128, R], F32, tag=f"zl{v}", name=f"zl{v}")
                    nc.vector.memset(zt[:], 0.0)
                    zlT.append(zt)

            # ---------------- gram A slots (input-only; overlaps AR) -----
            a_psums = {}

            def a_slot_emit(s, kg_lo, kg_hi):
                if s not in a_psums:
                    a_psums[s] = [
                        ps_a.tile([128, 512], F32, tag="aps", name=f"aps{s}_{j}")
                        for j in range(4)]
                psums = a_psums[s]
                for kg in range(kg_lo, kg_hi):
                    lt = gp.tile([128, KGRP, 512], BF16, tag="gal",
                                 name=f"gal{s}_{kg}")
                    nc.sync.dma_start(
                        lt[:],
                        ga_l.ap()[s, kg * KGRP * 128:(kg + 1) * KGRP * 128, :]
                        .rearrange("(k p) f -> p k f", p=128))
                    rt = gp.tile([128, KGRP, 512], BF16, tag="gar",
                                 name=f"gar{s}_{kg}")
                    nc.sync.dma_start(
                        rt[:],
                        ga_r.ap()[s, kg * KGRP * 128:(kg + 1) * KGRP * 128, :]
                        .rearrange("(k p) f -> p k f", p=128))
                    for kk in range(KGRP):
                        first = kg == 0 and kk == 0
                        last = kg == KB // KGRP - 1 and kk == KGRP - 1
                        for j in range(4):
                            nc.tensor.matmul(
                                psums[j][:], lt[:, kk, j * 128:(j + 1) * 128],
                                rt[:, kk, :], start=first, stop=last)
                if kg_hi == KB // KGRP:
                    for j in range(4):
                        nc.scalar.activation(
                            trash[:], psums[j][:], AF.Square,
                            accum_out=acc[:, 4 * s + j:4 * s + j + 1])

            # slot 0 right after the encoders: covers the stats-AllReduce
            # latency; remaining slots are emitted after the BN-dependent
            # PE work so they cover the z-AllGather + B-panel prefetch.
            if GA:
                a_slot_emit(0, 0, KB // KGRP)

            # ---------------- BN stats AllReduce ----------------
            st_in = dram.tile([128, 2 * NV], F32, tag="st_in")
            st_out = dram.tile([128, 2 * NV], F32, tag="st_out",
                               addr_space="Shared")
            statsg = pp.tile([128, 2 * NV], F32, tag="statsg")
            st_dma = None
            if AR:
                st_dma = nc.gpsimd.dma_start(st_in[:], stats[:])
                nc.gpsimd.collective_compute(
                    "AllReduce", ALU.add,
                    ins=[st_in.opt()], outs=[st_out.opt()],
                    replica_groups=[list(range(NCORES))])
                nc.gpsimd.dma_start(statsg[:], st_out[:])
            else:
                nc.vector.tensor_scalar_mul(statsg[:], stats[:], 8.0)

            # ---------------- BN apply + fused z ----------------
            mus = pp.tile([128, NV], F32, tag="mus")
            vart = pp.tile([128, NV], F32, tag="vart")
            nc.vector.tensor_scalar_mul(mus[:], statsg[:, 0:NV], 1.0 / N)
            nc.vector.tensor_scalar_mul(vart[:], statsg[:, NV:2 * NV], 1.0 / N)
            mu2 = pp.tile([128, NV], F32, tag="mu2")
            nc.vector.tensor_tensor(mu2[:], mus[:], mus[:], ALU.mult)
            nc.vector.tensor_tensor(vart[:], vart[:], mu2[:], ALU.subtract)
            stdt = pp.tile([128, NV], F32, tag="stdt")
            nc.scalar.activation(stdt[:], vart[:], AF.Sqrt, bias=epsc)
            invstd = pp.tile([128, NV], F32, tag="invstd")
            nc.vector.reciprocal(invstd[:], stdt[:])
            gall = pp.tile([128, NV], F32, tag="gall")
            ball = pp.tile([128, NV], F32, tag="ball")
            for v in range(NV):
                nc.vector.tensor_copy(gall[:, v:v + 1], gamc[v])
                nc.vector.tensor_copy(ball[:, v:v + 1], betc[v])
            scl = pp.tile([128, NV], F32, tag="scl")
            nc.vector.tensor_tensor(scl[:], gall[:], invstd[:], ALU.mult)
            shf = pp.tile([128, NV], F32, tag="shf")
            nc.vector.tensor_tensor(shf[:], mus[:], scl[:], ALU.mult)
            nc.vector.tensor_tensor(shf[:], ball[:], shf[:], ALU.subtract)

            ziT = []
            for v in range(NV):
                zi = pp.tile([128, R], F32, tag=f"zi{v}")
                nc.scalar.activation(zi[:], zlT[v][:], AF.Identity,
                                     bias=shf[:, v:v + 1], scale=scl[:, v:v + 1])
                ziT.append(zi)
            zT = pp.tile([128, R], F32, tag="zT")
            ztmp = pp.tile([128, R], F32, tag="ztmp")
            nc.vector.tensor_tensor(zT[:], ziT[0][:], nwb[0], ALU.mult)
            for v in range(1, NV):
                nc.vector.tensor_tensor(ztmp[:], ziT[v][:], nwb[v], ALU.mult)
                nc.vector.tensor_tensor(zT[:], zT[:], ztmp[:], ALU.add)

            # ---------------- natural-layout outputs + z AllGather -------
            zag_in = dram.tile([R, NZ], F32, tag="zag_in")
            zag_out = dram.tile([N, NZ], F32, tag="zag_out", addr_space="Shared")
            znat = sp.tile([128, RB, 128], F32, tag="znat", name="znat")
            for j in range(RB):
                pst = ps_sm.tile([128, 128], F32, tag="sm", name=f"ztp{j}")
                nc.tensor.transpose(pst[:], zT[:, j * 128:(j + 1) * 128], ident[:])
                nc.vector.tensor_copy(znat[:, j, :], pst[:])
            nc.gpsimd.dma_start(zo.ap().rearrange("(j p) f -> p j f", p=128), znat[:])
            nc.sync.dma_start(zag_in.rearrange("(j p) f -> p j f", p=128), znat[:])
            if AG:
                nc.gpsimd.collective_compute(
                    "AllGather", ALU.bypass,
                    ins=[zag_in.opt()], outs=[zag_out.opt()],
                    replica_groups=[list(range(NCORES))])
            for v in range(NV):
                zsnat = sp.tile([128, RB, 128], F32, tag="zsnat", name=f"zsnat{v}")
                for j in range(RB):
                    pst = ps_sm.tile([128, 128], F32, tag="sm", name=f"zstp{v}_{j}")
                    nc.tensor.transpose(pst[:], ziT[v][:, j * 128:(j + 1) * 128],
                                        ident[:])
                    nc.vector.tensor_copy(zsnat[:, j, :], pst[:])
                nc.gpsimd.dma_start(
                    zso.ap()[v].rearrange("(j p) f -> p j f", p=128), zsnat[:])

            # ---------------- reg heads (sigmoid(relu(z) W + b)) ---------
            def reg_head(zin, out_ap, label):
                rz = sp.tile([128, R], BF16, tag="reluz", name=f"rz_{label}")
                nc.scalar.activation(rz[:], zin[:], AF.Relu)
                psum = ps_sm.tile([NL, R], F32, tag="sm", name=f"regp_{label}")
                nc.tensor.matmul(psum[:], regw_t[:], rz[:], start=True, stop=True)
                yt = sp.tile([NL, R], F32, tag="yT", name=f"yt_{label}")
                nc.scalar.activation(yt[:], psum[:], AF.Sigmoid,
                                     bias=regb_t)
                ynat = sp.tile([128, RB, NL], F32, tag="ynat", name=f"yn_{label}")
                for j in range(RB):
                    pst = ps_sm.tile([128, NL], F32, tag="sm",
                                     name=f"ytp_{label}_{j}")
                    nc.tensor.matmul(pst[:], yt[:, j * 128:(j + 1) * 128],
                                     ident[:NL, :NL], is_transpose=True,
                                     start=True, stop=True)
                    nc.vector.tensor_copy(ynat[:, j, :], pst[:])
                nc.gpsimd.dma_start(
                    out_ap.rearrange("(j p) f -> p j f", p=128), ynat[:])

            reg_head(zT, yo.ap(), "y")
            for v in range(NV):
                reg_head(ziT[v], yspo.ap()[v], f"ysp{v}")

            if GA:
                for s in range(1, NASLOT):
                    a_slot_emit(s, 0, KB // KGRP)

            # ---------------- gathered Z: rz, scaled tiles, B & C --------
            rz2 = pp.tile([128, KB], F32, tag="rz2")
            zgt = zgf.tile([128, KB, 128], F32, tag="zg")
            if BC:
                nc.gpsimd.dma_start(
                    zgt[:], zag_out.rearrange("(k p) f -> p k f", p=128))
                for kb in range(KB):
                    nc.scalar.activation(trash[:, :128], zgt[:, kb, :], AF.Square,
                                         accum_out=rz2[:, kb:kb + 1])
            else:
                nc.vector.memset(rz2[:], 1.0)
            rzt = pp.tile([128, KB], F32, tag="rzt")
            nc.scalar.activation(rzt[:], rz2[:], AF.Sqrt)
            nc.vector.tensor_scalar_max(rzt[:], rzt[:], 1e-12)
            rzinv = pp.tile([128, KB], F32, tag="rzinv")
            nc.vector.reciprocal(rzinv[:], rzt[:])

            combB = [pp.tile([128, KB], F32, tag=f"combB{s}", name=f"combB{s}")
                     for s in range(NBSLOT)]
            for s in range(NBSLOT):
                nc.vector.tensor_tensor(combB[s][:], bw_t[s], rzinv[:],
                                        ALU.mult)
            combC = pp.tile([128, KB], F32, tag="combC")
            nc.vector.tensor_tensor(combC[:], cw_t, rzinv[:], ALU.mult)

            # B slots: psum [NZ, 512] = sum_kb (D_v Z)[kb]^T @ (D_u X)[kb, chunk]
            for s in range(NBSLOT if BC else 0):
                psum = ps_a.tile([128, 512], F32, tag="aps", name=f"bps{s}")
                for kg in range(KB // KGRP):
                    rt = gpb.tile([128, KGRP, 512], BF16, tag="gbr")
                    nc.scalar.dma_start(
                        rt[:],
                        gb_r.ap()[s, kg * KGRP * 128:(kg + 1) * KGRP * 128, :]
                        .rearrange("(k p) f -> p k f", p=128))
                    for kk in range(KGRP):
                        kb = kg * KGRP + kk
                        sz = zgp.tile([128, 128], BF16, tag="szb")
                        nc.vector.tensor_scalar_mul(sz[:], zgt[:, kb, :],
                                                    combB[s][:, kb:kb + 1])
                        nc.tensor.matmul(psum[:], sz[:], rt[:, kk, :],
                                         start=(kb == 0), stop=(kb == KB - 1))
                nc.scalar.activation(trash[:], psum[:], AF.Square,
                                     accum_out=acc[:, 16 + s:17 + s])

            # C: psum [NZ, NZ] = sum_kb (D_v Z)[kb]^T @ (D_v Z)[kb]
            if BC:
                psum = ps_sm.tile([128, 128], F32, tag="sm", name="cps")
                for kb in range(KB):
                    sc = zgp.tile([128, 128], BF16, tag="szc")
                    nc.vector.tensor_scalar_mul(sc[:], zgt[:, kb, :],
                                                combC[:, kb:kb + 1])
                    nc.tensor.matmul(psum[:], sc[:], sc[:],
                                     start=(kb == 0), stop=(kb == KB - 1))
                nc.scalar.activation(trash[:, :128], psum[:], AF.Square,
                                     accum_out=acc[:, 18:19])

            # ---------------- loss partial ----------------
            ra = pp.tile([128, 1], F32, tag="ra")
            rb = pp.tile([128, 1], F32, tag="rb")
            nc.vector.tensor_reduce(ra[:], acc[:, 0:16], mybir.AxisListType.X,
                                    ALU.add)
            nc.vector.tensor_reduce(rb[:], acc[:, 16:18], mybir.AxisListType.X,
                                    ALU.add)
            nc.vector.tensor_scalar_mul(rb[:], rb[:], -2.0)
            nc.vector.tensor_tensor(ra[:], ra[:], rb[:], ALU.add)
            nc.vector.tensor_tensor(ra[:], ra[:], acc[:, 18:19], ALU.add)
            lt = pp.tile([128, 1], F32, tag="lossf")
            nc.vector.tensor_copy(lt[:], ra[:])
            nc.sync.dma_start(lossp.ap()[:, :], lt[:])

    nc.compile()
    return nc


_NC = None


def _get_nc():
    global _NC
    if _NC is None:
        _NC = build_nc()
    return _NC


def _prep_in_maps(inputs):
    bf = ml_dtypes.bfloat16
    xs = [np.ascontiguousarray(np.asarray(inputs[f"x{i}"], dtype=np.float32))
          for i in range(NV)]
    we = np.asarray(inputs["we"], dtype=np.float32)
    p = inputs["params"]
    enc = p["enc"]
    var = np.asarray(p["variables"], dtype=np.float32)

    we_s = we * var[None, :]
    nw = we_s / we_s.sum(axis=1, keepdims=True)
    u = []
    for i in range(NV):
        r = np.sqrt((xs[i] ** 2).sum(axis=1))
        u.append(we_s[:, i] / np.maximum(r, 1e-12))

    def swz(w, kp, mp):
        # -> [mp//128, 128, kp]: out[m, p, k*128+f] = w_pad[k*128+p, m*128+f]
        k, m = w.shape
        out = np.zeros((kp, mp), np.float32)
        out[:k, :m] = np.asarray(w, np.float32)
        out = out.reshape(kp // 128, 128, mp // 128, 128)
        out = out.transpose(2, 1, 0, 3).reshape(mp // 128, 128, kp)
        return np.ascontiguousarray(out).astype(bf)

    def padv(b, n):
        out = np.zeros((n,), np.float32)
        out[: b.shape[0]] = np.asarray(b, np.float32)
        return out

    shared = {}
    for v in range(NV):
        e = enc[v]
        shared[f"w1_{v}"] = swz(e["W1"], D[v], D0P[v])
        shared[f"w2_{v}"] = swz(e["W2"], D0P[v], HIDP)
        shared[f"wz_{v}"] = swz(e["Wz"], HIDP, 128)
    shared["regw"] = np.asarray(p["reg_W"], np.float32).astype(bf)

    CCOLS = sum(D0P[v] // 128 + HIDP // 128 + 3 for v in range(NV)) + 3 * KB + 2
    # bw/cw columns are per-core; consts base (weights/biases) is shared
    cbase = np.zeros((128, CCOLS), np.float32)
    off = 0
    for v in range(NV):
        e = enc[v]
        nb1 = D0P[v] // 128
        cbase[:, off:off + nb1] = padv(e["b1"], D0P[v]).reshape(nb1, 128).T
        off += nb1
        nb2 = HIDP // 128
        cbase[:, off:off + nb2] = padv(e["b2"], HIDP).reshape(nb2, 128).T
        off += nb2
        cbase[:, off] = padv(e["bz"], NZ)
        cbase[:, off + 1] = np.asarray(e["gamma"], np.float32)
        cbase[:, off + 2] = np.asarray(e["beta"], np.float32)
        off += 3
    bw_off = off
    cw_off = off + NBSLOT * KB
    regb_off = cw_off + KB
    cbase[:NL, regb_off] = np.asarray(p["reg_b"], np.float32)
    cbase[:, regb_off + 1] = BN_EPS

    sqrt2 = np.float32(math.sqrt(2.0))
    in_maps = []
    for c in range(NCORES):
        rows = slice(c * R, (c + 1) * R)
        m = dict(shared)
        for v in range(NV):
            m[f"xt{v}"] = np.ascontiguousarray(xs[v][rows].T).astype(bf)
        m["nwl"] = np.ascontiguousarray(nw[rows].T)

        gal = np.zeros((NASLOT, N, 512), bf)
        gar = np.zeros((NASLOT, N, 512), bf)
        for s in range(NASLOT):
            idx = c * NASLOT + s
            if idx >= len(A_PIECES):
                continue
            v, cs, w, ms = A_PIECES[idx]
            uw = u[v][:, None]
            fac = sqrt2 if ms < cs else np.float32(1.0)
            lw = min(512, D[v] - ms)
            gal[s, :, :lw] = (xs[v][:, ms:ms + lw] * (uw * fac)).astype(bf)
            gar[s, :, :w] = (xs[v][:, cs:cs + w] * uw).astype(bf)
        m["ga_l"] = gal
        m["ga_r"] = gar

        gbr = np.zeros((NBSLOT, N, 512), bf)
        cc = cbase.copy()
        for s in range(NBSLOT):
            v, cs, w = B_UNITS[c * NBSLOT + s]
            gbr[s, :, :w] = (xs[v][:, cs:cs + w] * u[v][:, None]).astype(bf)
            cc[:, bw_off + s * KB:bw_off + (s + 1) * KB] = \
                we_s[:, v].reshape(KB, 128).T
        m["gb_r"] = gbr
        if c < NV:
            cc[:, cw_off:cw_off + KB] = we_s[:, c].reshape(KB, 128).T
        m["consts"] = cc
        in_maps.append(m)
    return in_maps


def kernel(**inputs):
    nc = _get_nc()
    in_maps = _prep_in_maps(inputs)
    res = run_bass_kernel_spmd(nc, in_maps, core_ids=list(range(NCORES)))
    r = res.results
    yL = np.concatenate([r[c]["yo"] for c in range(NCORES)], axis=0)
    z = np.concatenate([r[c]["zo"] for c in range(NCORES)], axis=0)
    zs = np.concatenate([r[c]["zso"] for c in range(NCORES)], axis=1)
    ysp = np.concatenate([r[c]["yspo"] for c in range(NCORES)], axis=1)
    total = np.float64(0.0)
    for c in range(NCORES):
        total += np.float64(r[c]["lossp"].sum())
    loss = np.float32(total / (float(N) * float(N)) / 6.0)
    return (yL, z, zs, ysp, loss)


# revision 26
# speedup vs baseline: 1.0305x; 1.0305x over previous
"""Trainium2 Bass kernel for nn_AE_89343909691943 (multi-view AE loss_fn).

8-core SPMD strategy (data-parallel over the N=4096 sample axis, 512 rows/core):
  * 6 per-view encoder MLPs run row-sharded in transposed-activation layout
    (features on partitions), so bias+ReLU fuse into PSUM evacuation and
    BatchNorm stats are per-partition reductions.
  * BN batch stats via a tiny AllReduce; fused z via an AllGather of z.
  * The N x N similarity matrices are never materialized.  The loss
      loss_i = mean((w (sims_i - sim2) w)^2)
    is computed exactly as
      (||X^T D_{u^2} X||_F^2 - 2||X^T D_{uv} Z||_F^2 + ||Z^T D_{v^2} Z||_F^2)/N^2
    with u = w/||x_row||, v = w/||z_row||, which shrinks the gram work from
    O(N^2 d) to O(N d^2) and keeps everything on the TensorEngine.
  * All GEMMs run in bf16 with fp32 PSUM accumulation (validated ~0.5% worst
    output error, loss error ~1e-5).
  * DMAs are batched (one instruction per weight m-panel / k-group) to keep
    the Sync sequencer's descriptor generation off the critical path.
"""

import sys

for _p in ("/opt/trn_rl_repo", "/root/.axon_site/_ro/trn_rl_repo"):
    if _p not in sys.path:
        sys.path.insert(0, _p)

import math

import ml_dtypes
import numpy as np

import concourse.bass as bass
import concourse.mybir as mybir
import concourse.tile as tile
from concourse import bacc
from concourse.bass_utils import run_bass_kernel_spmd
from concourse.masks import make_identity

BF16 = mybir.dt.bfloat16
F32 = mybir.dt.float32
AF = mybir.ActivationFunctionType
ALU = mybir.AluOpType

N = 4096
NCORES = 8
R = N // NCORES  # 512 rows per core
NZ = 128
NL = 80
NV = 6
D = [1024, 1536, 2048, 1280, 896, 1024]
D0 = [819, 1229, 1638, 1024, 717, 819]
D0P = [896, 1280, 1664, 1024, 768, 896]
HID = 1500
HIDP = 1536
BN_EPS = 1e-5
KB = N // 128  # 32 row-blocks of 128
RB = R // 128  # 4 local row-blocks
KGRP = 4  # gram k-blocks fetched per DMA

NASLOT = 4  # gram-A pieces per core (uniform across cores)
NBSLOT = 2  # gram-B (cross term) units per core


def _a_pieces():
    """(view, chunk_start, chunk_width, m_start) gram-A pieces.

    A_i = (D_u X_i)^T (D_u X_i) is computed in 512-wide column chunks; for
    each chunk only m <= chunk_end 512-wide lhsT pieces are needed (symmetric
    matrix; strictly-below pieces get a sqrt(2) factor folded into the host
    scaling so ||.||^2 counts them twice)."""
    pieces = []
    for v in range(NV):
        for cs in range(0, D[v], 512):
            w = min(512, D[v] - cs)
            for ms in range(0, cs + 1, 512):
                pieces.append((v, cs, w, ms))
    return pieces


def _b_units():
    units = []
    for v in range(NV):
        for cs in range(0, D[v], 512):
            units.append((v, cs, min(512, D[v] - cs)))
    return units


A_PIECES = _a_pieces()  # 31 pieces -> 32 slots (one zero slot)
B_UNITS = _b_units()  # 16 units -> exactly 2 per core
assert len(A_PIECES) <= NASLOT * NCORES
assert len(B_UNITS) == NBSLOT * NCORES


def build_nc(stages=frozenset({"enc", "ar", "a", "ag", "bc"})):
    nc = bacc.Bacc("TRN2", target_bir_lowering=False, debug=False,
                   num_devices=NCORES)

    ENC = "enc" in stages
    AR = "ar" in stages
    GA = "a" in stages
    AG = "ag" in stages
    BC = "bc" in stages

    # ---------------- parameters ----------------
    # weights are host-swizzled to [m_tile, 128, K*128]:
    #   w[m, p, k*128+f] = W_padded[k*128 + p, m*128 + f]
    xt = [nc.declare_dram_parameter(f"xt{v}", [128, D[v] // 128 * R], BF16,
                                    isOutput=False)
          for v in range(NV)]
    w1 = [nc.declare_dram_parameter(
        f"w1_{v}", [D0P[v] // 128, 128, D[v]], BF16, isOutput=False)
        for v in range(NV)]
    w2 = [nc.declare_dram_parameter(
        f"w2_{v}", [HIDP // 128, 128, D0P[v]], BF16, isOutput=False)
        for v in range(NV)]
    wz = [nc.declare_dram_parameter(
        f"wz_{v}", [1, 128, HIDP], BF16, isOutput=False)
        for v in range(NV)]
    # all small per-partition constants packed into one [128, CCOLS] param:
    # cols: per view [b1c(D0P/128) b2c(12) bz gam bet] then bw(2*32) cw(32)
    # regb(1) eps(1)
    CCOLS = sum(D0P[v] // 128 + HIDP // 128 + 3 for v in range(NV)) + 3 * KB + 2
    consts = nc.declare_dram_parameter("consts", [128, CCOLS], F32,
                                       isOutput=False)
    regw = nc.declare_dram_parameter("regw", [NZ, NL], BF16, isOutput=False)
    nwl = nc.declare_dram_parameter("nwl", [NV, R], F32, isOutput=False)
    ga_l = nc.declare_dram_parameter(
        "ga_l", [NASLOT, KB // KGRP, 128, KGRP * 512], BF16, isOutput=False)
    ga_r = nc.declare_dram_parameter(
        "ga_r", [NASLOT, KB // KGRP, 128, KGRP * 512], BF16, isOutput=False)
    gb_r = nc.declare_dram_parameter(
        "gb_r", [NBSLOT, KB // KGRP, 128, KGRP * 512], BF16, isOutput=False)

    yo = nc.declare_dram_parameter("yo", [R, NL], F32, isOutput=True)
    zo = nc.declare_dram_parameter("zo", [R, NZ], F32, isOutput=True)
    zso = nc.declare_dram_parameter("zso", [NV, R, NZ], F32, isOutput=True)
    yspo = nc.declare_dram_parameter("yspo", [NV, R, NL], F32, isOutput=True)
    lossp = nc.declare_dram_parameter("lossp", [128, 1], F32, isOutput=True)

    with tile.TileContext(nc) as tc:
        with (
            tc.tile_pool(name="const", bufs=1) as const,
            tc.tile_pool(name="persist", bufs=1) as pp,
            tc.tile_pool(name="small", bufs=2) as sp,
            tc.tile_pool(name="xr", bufs=2) as xr,
            tc.tile_pool(name="h1p", bufs=14) as h1p,
            tc.tile_pool(name="h2p", bufs=13) as h2p,
            tc.tile_pool(name="wp", bufs=3) as wp,
            tc.tile_pool(name="gp", bufs=3) as gp,
            tc.tile_pool(name="gpb", bufs=5) as gpb,
            tc.tile_pool(name="zgp", bufs=4) as zgp,
            tc.tile_pool(name="zgf", bufs=1) as zgf,
            tc.tile_pool(name="ps_enc", bufs=2, space="PSUM") as ps_enc,
            tc.tile_pool(name="ps_a", bufs=4, space="PSUM") as ps_a,
            tc.tile_pool(name="ps_sm", bufs=2, space="PSUM") as ps_sm,
            tc.tile_pool(name="dram", bufs=1, space="DRAM") as dram,
        ):
            # ---------------- constants / small loads ----------------
            ident = const.tile([128, 128], F32)
            make_identity(nc, ident)

            call = const.tile([128, CCOLS], F32, tag="call")
            nc.gpsimd.dma_start(call[:], consts.ap())
            b1c, b2c, bzc, gamc, betc = [], [], [], [], []
            off = 0
            for v in range(NV):
                b1c.append(call[:, off:off + D0P[v] // 128])
                off += D0P[v] // 128
                b2c.append(call[:, off:off + HIDP // 128])
                off += HIDP // 128
                bzc.append(call[:, off:off + 1])
                gamc.append(call[:, off + 1:off + 2])
                betc.append(call[:, off + 2:off + 3])
                off += 3
            bw_t = [call[:, off + s * KB:off + (s + 1) * KB]
                    for s in range(NBSLOT)]
            off += NBSLOT * KB
            cw_t = call[:, off:off + KB]
            off += KB
            regb_t = call[:NL, off:off + 1]
            epsc = call[:, off + 1:off + 2]
            regw_t = const.tile([128, NL], BF16, tag="regw")
            nc.gpsimd.dma_start(regw_t[:], regw.ap())
            nwb_all = const.tile([128, NV, R], F32, tag="nwb")
            nc.gpsimd.dma_start(
                nwb_all[:],
                nwl.ap().rearrange("(o v) f -> o v f", o=1)
                .to_broadcast((128, NV, R)))
            nwb = [nwb_all[:, v, :] for v in range(NV)]
            stats = pp.tile([128, 2 * NV], F32, tag="stats")
            nc.vector.memset(stats[:], 0.0)
            acc = pp.tile([128, 24], F32, tag="acc")
            nc.vector.memset(acc[:], 0.0)
            trash = pp.tile([128, 512], BF16, tag="trash")

            # ---------------- encoders (transposed activations) ---------
            zlT = []
            for v in range(NV if ENC else 0):
                kd = D[v] // 128
                km = D0P[v] // 128
                kh = HIDP // 128
                xtt = xr.tile([128, 16, R], BF16, tag="xt")
                nc.scalar.dma_start(
                    xtt[:, :kd, :], xt[v].ap().rearrange("p (k f) -> p k f", k=kd))
                # h1T = relu(W1^T x^T + b1)
                h1t = []
                for m in range(km):
                    wt = wp.tile([128, 2048], BF16, tag="w")
                    nc.sync.dma_start(wt[:, :D[v]], w1[v].ap()[m])
                    psum = ps_enc.tile([128, R], F32, tag="enc")
                    for k in range(kd):
                        nc.tensor.matmul(psum[:], wt[:, k * 128:(k + 1) * 128],
                                         xtt[:, k, :],
                                         start=(k == 0), stop=(k == kd - 1))
                    h = h1p.tile([128, R], BF16, tag="h1")
                    nc.scalar.activation(h[:], psum[:], AF.Relu,
                                         bias=b1c[v][:, m:m + 1])
                    h1t.append(h)
                # h2T = relu(W2^T h1T + b2)
                h2t = []
                for m in range(kh):
                    wt = wp.tile([128, 2048], BF16, tag="w")
                    nc.sync.dma_start(wt[:, :D0P[v]], w2[v].ap()[m])
                    psum = ps_enc.tile([128, R], F32, tag="enc")
                    for k in range(km):
                        nc.tensor.matmul(psum[:], wt[:, k * 128:(k + 1) * 128],
                                         h1t[k][:],
                                         start=(k == 0), stop=(k == km - 1))
                    h = h2p.tile([128, R], BF16, tag="h2")
                    nc.scalar.activation(h[:], psum[:], AF.Relu,
                                         bias=b2c[v][:, m:m + 1])
                    h2t.append(h)
                # zlT = Wz^T h2T + bz  (fp32, keep; stats fused)
                wt = wp.tile([128, 2048], BF16, tag="w")
                nc.sync.dma_start(wt[:, :HIDP], wz[v].ap()[0])
                psum = ps_enc.tile([128, R], F32, tag="enc")
                for k in range(kh):
                    nc.tensor.matmul(psum[:], wt[:, k * 128:(k + 1) * 128],
                                     h2t[k][:],
                                     start=(k == 0), stop=(k == kh - 1))
                zt = pp.tile([128, R], F32, tag=f"zl{v}")
                ssum = pp.tile([128, 1], F32, tag=f"ssum{v}", name=f"ssum{v}")
                ssq = pp.tile([128, 1], F32, tag=f"ssq{v}", name=f"ssq{v}")
                nc.scalar.activation(zt[:], psum[:], AF.Identity,
                                     bias=bzc[v], accum_out=ssum[:])
                nc.scalar.activation(trash[:], zt[:], AF.Square,
                                     accum_out=ssq[:])
                nc.vector.tensor_copy(stats[:, v:v + 1], ssum[:])
                last_stats_inst = nc.vector.tensor_copy(
                    stats[:, NV + v:NV + v + 1], ssq[:])
                zlT.append(zt)
            if not ENC:
                for v in range(NV):
                    zt = pp.tile([128, R], F32, tag=f"zl{v}", name=f"zl{v}")
                    nc.vector.memset(zt[:], 0.0)
                    zlT.append(zt)

            # ---------------- gram A slots (input-only; overlaps AR) -----
            a_psums = {}

            def a_slot_emit(s, kg_lo, kg_hi):
                if s not in a_psums:
                    a_psums[s] = [
                        ps_a.tile([128, 512], F32, tag="aps", name=f"aps{s}_{j}")
                        for j in range(4)]
                psums = a_psums[s]
                for kg in range(kg_lo, kg_hi):
                    lt = gp.tile([128, KGRP, 512], BF16, tag="gal",
                                 name=f"gal{s}_{kg}")
                    nc.sync.dma_start(
                        lt[:], ga_l.ap()[s, kg].rearrange(
                            "p (k f) -> p k f", k=KGRP))
                    rt = gp.tile([128, KGRP, 512], BF16, tag="gar",
                                 name=f"gar{s}_{kg}")
                    nc.sync.dma_start(
                        rt[:], ga_r.ap()[s, kg].rearrange(
                            "p (k f) -> p k f", k=KGRP))
                    for kk in range(KGRP):
                        first = kg == 0 and kk == 0
                        last = kg == KB // KGRP - 1 and kk == KGRP - 1
                        for j in range(4):
                            nc.tensor.matmul(
                                psums[j][:], lt[:, kk, j * 128:(j + 1) * 128],
                                rt[:, kk, :], start=first, stop=last)
                if kg_hi == KB // KGRP:
                    for j in range(4):
                        nc.scalar.activation(
                            trash[:], psums[j][:], AF.Square,
                            accum_out=acc[:, 4 * s + j:4 * s + j + 1])

            # slot 0 right after the encoders: covers the stats-AllReduce
            # latency; remaining slots are emitted after the BN-dependent
            # PE work so they cover the z-AllGather + B-panel prefetch.
            if GA:
                a_slot_emit(0, 0, KB // KGRP)

            # ---------------- BN stats AllReduce ----------------
            st_in = dram.tile([128, 2 * NV], F32, tag="st_in")
            st_out = dram.tile([128, 2 * NV], F32, tag="st_out",
                               addr_space="Shared")
            statsg = pp.tile([128, 2 * NV], F32, tag="statsg")
            st_dma = None
            if AR:
                st_dma = nc.gpsimd.dma_start(st_in[:], stats[:])
                nc.gpsimd.collective_compute(
                    "AllReduce", ALU.add,
                    ins=[st_in.opt()], outs=[st_out.opt()],
                    replica_groups=[list(range(NCORES))])
                nc.gpsimd.dma_start(statsg[:], st_out[:])
            else:
                nc.vector.tensor_scalar_mul(statsg[:], stats[:], 8.0)

            # ---------------- BN apply + fused z ----------------
            mus = pp.tile([128, NV], F32, tag="mus")
            vart = pp.tile([128, NV], F32, tag="vart")
            nc.vector.tensor_scalar_mul(mus[:], statsg[:, 0:NV], 1.0 / N)
            nc.vector.tensor_scalar_mul(vart[:], statsg[:, NV:2 * NV], 1.0 / N)
            mu2 = pp.tile([128, NV], F32, tag="mu2")
            nc.vector.tensor_tensor(mu2[:], mus[:], mus[:], ALU.mult)
            nc.vector.tensor_tensor(vart[:], vart[:], mu2[:], ALU.subtract)
            stdt = pp.tile([128, NV], F32, tag="stdt")
            nc.scalar.activation(stdt[:], vart[:], AF.Sqrt, bias=epsc)
            invstd = pp.tile([128, NV], F32, tag="invstd")
            nc.vector.reciprocal(invstd[:], stdt[:])
            gall = pp.tile([128, NV], F32, tag="gall")
            ball = pp.tile([128, NV], F32, tag="ball")
            for v in range(NV):
                nc.vector.tensor_copy(gall[:, v:v + 1], gamc[v])
                nc.vector.tensor_copy(ball[:, v:v + 1], betc[v])
            scl = pp.tile([128, NV], F32, tag="scl")
            nc.vector.tensor_tensor(scl[:], gall[:], invstd[:], ALU.mult)
            shf = pp.tile([128, NV], F32, tag="shf")
            nc.vector.tensor_tensor(shf[:], mus[:], scl[:], ALU.mult)
            nc.vector.tensor_tensor(shf[:], ball[:], shf[:], ALU.subtract)

            ziT = []
            for v in range(NV):
                zi = pp.tile([128, R], F32, tag=f"zi{v}")
                nc.scalar.activation(zi[:], zlT[v][:], AF.Identity,
                                     bias=shf[:, v:v + 1], scale=scl[:, v:v + 1])
                ziT.append(zi)
            zT = pp.tile([128, R], F32, tag="zT")
            ztmp = pp.tile([128, R], F32, tag="ztmp")
            nc.vector.tensor_tensor(zT[:], ziT[0][:], nwb[0], ALU.mult)
            for v in range(1, NV):
                nc.vector.tensor_tensor(ztmp[:], ziT[v][:], nwb[v], ALU.mult)
                nc.vector.tensor_tensor(zT[:], zT[:], ztmp[:], ALU.add)

            # ---------------- natural-layout outputs + z AllGather -------
            zag_in = dram.tile([R, NZ], F32, tag="zag_in")
            zag_out = dram.tile([N, NZ], F32, tag="zag_out", addr_space="Shared")
            znat = sp.tile([128, RB, 128], F32, tag="znat", name="znat")
            for j in range(RB):
                pst = ps_sm.tile([128, 128], F32, tag="sm", name=f"ztp{j}")
                nc.tensor.transpose(pst[:], zT[:, j * 128:(j + 1) * 128], ident[:])
                nc.vector.tensor_copy(znat[:, j, :], pst[:])
            nc.gpsimd.dma_start(zo.ap().rearrange("(j p) f -> p j f", p=128), znat[:])
            nc.sync.dma_start(zag_in.rearrange("(j p) f -> p j f", p=128), znat[:])
            if AG:
                nc.gpsimd.collective_compute(
                    "AllGather", ALU.bypass,
                    ins=[zag_in.opt()], outs=[zag_out.opt()],
                    replica_groups=[list(range(NCORES))])
            for v in range(NV):
                zsnat = sp.tile([128, RB, 128], F32, tag="zsnat", name=f"zsnat{v}")
                for j in range(RB):
                    pst = ps_sm.tile([128, 128], F32, tag="sm", name=f"zstp{v}_{j}")
                    nc.tensor.transpose(pst[:], ziT[v][:, j * 128:(j + 1) * 128],
                                        ident[:])
                    nc.vector.tensor_copy(zsnat[:, j, :], pst[:])
                nc.gpsimd.dma_start(
                    zso.ap()[v].rearrange("(j p) f -> p j f", p=128), zsnat[:])

            # ---------------- reg heads (sigmoid(relu(z) W + b)) ---------
            def reg_head(zin, out_ap, label):
                rz = sp.tile([128, R], BF16, tag="reluz", name=f"rz_{label}")
                nc.scalar.activation(rz[:], zin[:], AF.Relu)
                psum = ps_sm.tile([NL, R], F32, tag="sm", name=f"regp_{label}")
                nc.tensor.matmul(psum[:], regw_t[:], rz[:], start=True, stop=True)
                yt = sp.tile([NL, R], F32, tag="yT", name=f"yt_{label}")
                nc.scalar.activation(yt[:], psum[:], AF.Sigmoid,
                                     bias=regb_t)
                ynat = sp.tile([128, RB, NL], F32, tag="ynat", name=f"yn_{label}")
                for j in range(RB):
                    pst = ps_sm.tile([128, NL], F32, tag="sm",
                                     name=f"ytp_{label}_{j}")
                    nc.tensor.matmul(pst[:], yt[:, j * 128:(j + 1) * 128],
                                     ident[:NL, :NL], is_transpose=True,
                                     start=True, stop=True)
                    nc.vector.tensor_copy(ynat[:, j, :], pst[:])
                nc.gpsimd.dma_start(
                    out_ap.rearrange("(j p) f -> p j f", p=128), ynat[:])

            reg_head(zT, yo.ap(), "y")
            for v in range(NV):
                reg_head(ziT[v], yspo.ap()[v], f"ysp{v}")

            if GA:
                for s in range(1, NASLOT):
                    a_slot_emit(s, 0, KB // KGRP)

            # ---------------- gathered Z: rz, scaled tiles, B & C --------
            rz2 = pp.tile([128, KB], F32, tag="rz2")
            zgt = zgf.tile([128, KB, 128], F32, tag="zg")
            if BC:
                nc.gpsimd.dma_start(
                    zgt[:], zag_out.rearrange("(k p) f -> p k f", p=128))
                for kb in range(KB):
                    nc.scalar.activation(trash[:, :128], zgt[:, kb, :], AF.Square,
                                         accum_out=rz2[:, kb:kb + 1])
            else:
                nc.vector.memset(rz2[:], 1.0)
            rzt = pp.tile([128, KB], F32, tag="rzt")
            nc.scalar.activation(rzt[:], rz2[:], AF.Sqrt)
            nc.vector.tensor_scalar_max(rzt[:], rzt[:], 1e-12)
            rzinv = pp.tile([128, KB], F32, tag="rzinv")
            nc.vector.reciprocal(rzinv[:], rzt[:])

            combB = [pp.tile([128, KB], F32, tag=f"combB{s}", name=f"combB{s}")
                     for s in range(NBSLOT)]
            for s in range(NBSLOT):
                nc.vector.tensor_tensor(combB[s][:], bw_t[s], rzinv[:],
                                        ALU.mult)
            combC = pp.tile([128, KB], F32, tag="combC")
            nc.vector.tensor_tensor(combC[:], cw_t, rzinv[:], ALU.mult)

            # B slots: psum [NZ, 512] = sum_kb (D_v Z)[kb]^T @ (D_u X)[kb, chunk]
            for s in range(NBSLOT if BC else 0):
                psum = ps_a.tile([128, 512], F32, tag="aps", name=f"bps{s}")
                for kg in range(KB // KGRP):
                    rt = gpb.tile([128, KGRP, 512], BF16, tag="gbr")
                    nc.scalar.dma_start(
                        rt[:], gb_r.ap()[s, kg].rearrange(
                            "p (k f) -> p k f", k=KGRP))
                    for kk in range(KGRP):
                        kb = kg * KGRP + kk
                        sz = zgp.tile([128, 128], BF16, tag="szb")
                        nc.vector.tensor_scalar_mul(sz[:], zgt[:, kb, :],
                                                    combB[s][:, kb:kb + 1])
                        nc.tensor.matmul(psum[:], sz[:], rt[:, kk, :],
                                         start=(kb == 0), stop=(kb == KB - 1))
                nc.scalar.activation(trash[:], psum[:], AF.Square,
                                     accum_out=acc[:, 16 + s:17 + s])

            # C: psum [NZ, NZ] = sum_kb (D_v Z)[kb]^T @ (D_v Z)[kb]
            if BC:
                psum = ps_sm.tile([128, 128], F32, tag="sm", name="cps")
                for kb in range(KB):
                    sc = zgp.tile([128, 128], BF16, tag="szc")
                    nc.vector.tensor_scalar_mul(sc[:], zgt[:, kb, :],
                                                combC[:, kb:kb + 1])
                    nc.tensor.matmul(psum[:], sc[:], sc[:],
                                     start=(kb == 0), stop=(kb == KB - 1))
                nc.scalar.activation(trash[:, :128], psum[:], AF.Square,
                                     accum_out=acc[:, 18:19])

            # ---------------- loss partial ----------------
            ra = pp.tile([128, 1], F32, tag="ra")
            rb = pp.tile([128, 1], F32, tag="rb")
            nc.vector.tensor_reduce(ra[:], acc[:, 0:16], mybir.AxisListType.X,
                                    ALU.add)
            nc.vector.tensor_reduce(rb[:], acc[:, 16:18], mybir.AxisListType.X,
                                    ALU.add)
            nc.vector.tensor_scalar_mul(rb[:], rb[:], -2.0)
            nc.vector.tensor_tensor(ra[:], ra[:], rb[:], ALU.add)
            nc.vector.tensor_tensor(ra[:], ra[:], acc[:, 18:19], ALU.add)
            lt = pp.tile([128, 1], F32, tag="lossf")
            nc.vector.tensor_copy(lt[:], ra[:])
            nc.sync.dma_start(lossp.ap()[:, :], lt[:])

    nc.compile()
    return nc


_NC = None


def _get_nc():
    global _NC
    if _NC is None:
        _NC = build_nc()
    return _NC


def _prep_in_maps(inputs):
    bf = ml_dtypes.bfloat16
    xs = [np.ascontiguousarray(np.asarray(inputs[f"x{i}"], dtype=np.float32))
          for i in range(NV)]
    we = np.asarray(inputs["we"], dtype=np.float32)
    p = inputs["params"]
    enc = p["enc"]
    var = np.asarray(p["variables"], dtype=np.float32)

    we_s = we * var[None, :]
    nw = we_s / we_s.sum(axis=1, keepdims=True)
    u = []
    for i in range(NV):
        r = np.sqrt((xs[i] ** 2).sum(axis=1))
        u.append(we_s[:, i] / np.maximum(r, 1e-12))

    def swz(w, kp, mp):
        # -> [mp//128, 128, kp]: out[m, p, k*128+f] = w_pad[k*128+p, m*128+f]
        k, m = w.shape
        out = np.zeros((kp, mp), np.float32)
        out[:k, :m] = np.asarray(w, np.float32)
        out = out.reshape(kp // 128, 128, mp // 128, 128)
        out = out.transpose(2, 1, 0, 3).reshape(mp // 128, 128, kp)
        return np.ascontiguousarray(out).astype(bf)

    def padv(b, n):
        out = np.zeros((n,), np.float32)
        out[: b.shape[0]] = np.asarray(b, np.float32)
        return out

    shared = {}
    for v in range(NV):
        e = enc[v]
        shared[f"w1_{v}"] = swz(e["W1"], D[v], D0P[v])
        shared[f"w2_{v}"] = swz(e["W2"], D0P[v], HIDP)
        shared[f"wz_{v}"] = swz(e["Wz"], HIDP, 128)
    shared["regw"] = np.asarray(p["reg_W"], np.float32).astype(bf)

    CCOLS = sum(D0P[v] // 128 + HIDP // 128 + 3 for v in range(NV)) + 3 * KB + 2
    # bw/cw columns are per-core; consts base (weights/biases) is shared
    cbase = np.zeros((128, CCOLS), np.float32)
    off = 0
    for v in range(NV):
        e = enc[v]
        nb1 = D0P[v] // 128
        cbase[:, off:off + nb1] = padv(e["b1"], D0P[v]).reshape(nb1, 128).T
        off += nb1
        nb2 = HIDP // 128
        cbase[:, off:off + nb2] = padv(e["b2"], HIDP).reshape(nb2, 128).T
        off += nb2
        cbase[:, off] = padv(e["bz"], NZ)
        cbase[:, off + 1] = np.asarray(e["gamma"], np.float32)
        cbase[:, off + 2] = np.asarray(e["beta"], np.float32)
        off += 3
    bw_off = off
    cw_off = off + NBSLOT * KB
    regb_off = cw_off + KB
    cbase[:NL, regb_off] = np.asarray(p["reg_b"], np.float32)
    cbase[:, regb_off + 1] = BN_EPS

    sqrt2 = np.float32(math.sqrt(2.0))
    in_maps = []
    for c in range(NCORES):
        rows = slice(c * R, (c + 1) * R)
        m = dict(shared)
        for v in range(NV):
            kd = D[v] // 128
            xtv = xs[v][rows].T.reshape(kd, 128, R).transpose(1, 0, 2)
            m[f"xt{v}"] = np.ascontiguousarray(
                xtv.reshape(128, kd * R)).astype(bf)
        m["nwl"] = np.ascontiguousarray(nw[rows].T)

        gal = np.zeros((NASLOT, N, 512), bf)
        gar = np.zeros((NASLOT, N, 512), bf)
        KGG = KB // KGRP
        for s in range(NASLOT):
            idx = c * NASLOT + s
            if idx >= len(A_PIECES):
                continue
            v, cs, w, ms = A_PIECES[idx]
            uw = u[v][:, None]
            fac = sqrt2 if ms < cs else np.float32(1.0)
            lw = min(512, D[v] - ms)
            gal[s, :, :lw] = (xs[v][:, ms:ms + lw] * (uw * fac)).astype(bf)
            gar[s, :, :w] = (xs[v][:, cs:cs + w] * uw).astype(bf)
        def gsw(g, nslot):
            # [S, N, 512] -> [S, KGG, 128, KGRP*512]
            g = g.reshape(nslot, KGG, KGRP, 128, 512).transpose(0, 1, 3, 2, 4)
            return np.ascontiguousarray(
                g.reshape(nslot, KGG, 128, KGRP * 512))
        m["ga_l"] = gsw(gal, NASLOT)
        m["ga_r"] = gsw(gar, NASLOT)

        gbr = np.zeros((NBSLOT, N, 512), bf)
        cc = cbase.copy()
        for s in range(NBSLOT):
            v, cs, w = B_UNITS[c * NBSLOT + s]
            gbr[s, :, :w] = (xs[v][:, cs:cs + w] * u[v][:, None]).astype(bf)
            cc[:, bw_off + s * KB:bw_off + (s + 1) * KB] = \
                we_s[:, v].reshape(KB, 128).T
        m["gb_r"] = gsw(gbr, NBSLOT)
        if c < NV:
            cc[:, cw_off:cw_off + KB] = we_s[:, c].reshape(KB, 128).T
        m["consts"] = cc
        in_maps.append(m)
    return in_maps


def kernel(**inputs):
    nc = _get_nc()
    in_maps = _prep_in_maps(inputs)
    res = run_bass_kernel_spmd(nc, in_maps, core_ids=list(range(NCORES)))
    r = res.results
    yL = np.concatenate([r[c]["yo"] for c in range(NCORES)], axis=0)
    z = np.concatenate([r[c]["zo"] for c in range(NCORES)], axis=0)
    zs = np.concatenate([r[c]["zso"] for c in range(NCORES)], axis=1)
    ysp = np.concatenate([r[c]["yspo"] for c in range(NCORES)], axis=1)
    total = np.float64(0.0)
    for c in range(NCORES):
        total += np.float64(r[c]["lossp"].sum())
    loss = np.float32(total / (float(N) * float(N)) / 6.0)
    return (yL, z, zs, ysp, loss)


# revision 27
# speedup vs baseline: 1.0756x; 1.0437x over previous
"""Trainium2 Bass kernel for nn_AE_89343909691943 (multi-view AE loss_fn).

8-core SPMD strategy (data-parallel over the N=4096 sample axis, 512 rows/core):
  * 6 per-view encoder MLPs run row-sharded in transposed-activation layout
    (features on partitions), so bias+ReLU fuse into PSUM evacuation and
    BatchNorm stats are per-partition reductions.
  * BN batch stats via a tiny AllReduce; fused z via an AllGather of z.
  * The N x N similarity matrices are never materialized.  The loss
      loss_i = mean((w (sims_i - sim2) w)^2)
    is computed exactly as
      (||X^T D_{u^2} X||_F^2 - 2||X^T D_{uv} Z||_F^2 + ||Z^T D_{v^2} Z||_F^2)/N^2
    with u = w/||x_row||, v = w/||z_row||, which shrinks the gram work from
    O(N^2 d) to O(N d^2) and keeps everything on the TensorEngine.
  * All GEMMs run in bf16 with fp32 PSUM accumulation (validated ~0.5% worst
    output error, loss error ~1e-5).
  * DMAs are batched (one instruction per weight m-panel / k-group) to keep
    the Sync sequencer's descriptor generation off the critical path.
"""

import sys

for _p in ("/opt/trn_rl_repo", "/root/.axon_site/_ro/trn_rl_repo"):
    if _p not in sys.path:
        sys.path.insert(0, _p)

import math

import ml_dtypes
import numpy as np

import concourse.bass as bass
import concourse.mybir as mybir
import concourse.tile as tile
from concourse import bacc
from concourse.bass_utils import run_bass_kernel_spmd
from concourse.masks import make_identity

BF16 = mybir.dt.bfloat16
FP8 = mybir.dt.float8e4
FP8_SCALE = 64.0
SZ_SCALE = 16.0
F32 = mybir.dt.float32
AF = mybir.ActivationFunctionType
ALU = mybir.AluOpType

N = 4096
NCORES = 8
R = N // NCORES  # 512 rows per core
NZ = 128
NL = 80
NV = 6
D = [1024, 1536, 2048, 1280, 896, 1024]
D0 = [819, 1229, 1638, 1024, 717, 819]
D0P = [896, 1280, 1664, 1024, 768, 896]
HID = 1500
HIDP = 1536
BN_EPS = 1e-5
KB = N // 128  # 32 row-blocks of 128
RB = R // 128  # 4 local row-blocks
KGRP = 4  # gram k-blocks fetched per DMA

NASLOT = 4  # gram-A pieces per core (uniform across cores)
NBSLOT = 2  # gram-B (cross term) units per core


def _a_pieces():
    """(view, chunk_start, chunk_width, m_start) gram-A pieces.

    A_i = (D_u X_i)^T (D_u X_i) is computed in 512-wide column chunks; for
    each chunk only m <= chunk_end 512-wide lhsT pieces are needed (symmetric
    matrix; strictly-below pieces get a sqrt(2) factor folded into the host
    scaling so ||.||^2 counts them twice)."""
    pieces = []
    for v in range(NV):
        for cs in range(0, D[v], 512):
            w = min(512, D[v] - cs)
            for ms in range(0, cs + 1, 512):
                pieces.append((v, cs, w, ms))
    return pieces


def _b_units():
    units = []
    for v in range(NV):
        for cs in range(0, D[v], 512):
            units.append((v, cs, min(512, D[v] - cs)))
    return units


A_PIECES = _a_pieces()  # 31 pieces -> 32 slots (one zero slot)
B_UNITS = _b_units()  # 16 units -> exactly 2 per core
assert len(A_PIECES) <= NASLOT * NCORES
assert len(B_UNITS) == NBSLOT * NCORES


def build_nc(stages=frozenset({"enc", "ar", "a", "ag", "bc"})):
    nc = bacc.Bacc("TRN2", target_bir_lowering=False, debug=False,
                   num_devices=NCORES)

    ENC = "enc" in stages
    AR = "ar" in stages
    GA = "a" in stages
    AG = "ag" in stages
    BC = "bc" in stages

    # ---------------- parameters ----------------
    # weights are host-swizzled to [m_tile, 128, K*128]:
    #   w[m, p, k*128+f] = W_padded[k*128 + p, m*128 + f]
    xt = [nc.declare_dram_parameter(f"xt{v}", [128, D[v] // 128 * R], BF16,
                                    isOutput=False)
          for v in range(NV)]
    w1 = [nc.declare_dram_parameter(
        f"w1_{v}", [D0P[v] // 128, 128, D[v]], BF16, isOutput=False)
        for v in range(NV)]
    w2 = [nc.declare_dram_parameter(
        f"w2_{v}", [HIDP // 128, 128, D0P[v]], BF16, isOutput=False)
        for v in range(NV)]
    wz = [nc.declare_dram_parameter(
        f"wz_{v}", [1, 128, HIDP], BF16, isOutput=False)
        for v in range(NV)]
    # all small per-partition constants packed into one [128, CCOLS] param:
    # cols: per view [b1c(D0P/128) b2c(12) bz gam bet] then bw(2*32) cw(32)
    # regb(1) eps(1)
    CCOLS = sum(D0P[v] // 128 + HIDP // 128 + 3 for v in range(NV)) + 3 * KB + 2
    consts = nc.declare_dram_parameter("consts", [128, CCOLS], F32,
                                       isOutput=False)
    regw = nc.declare_dram_parameter("regw", [NZ, NL], BF16, isOutput=False)
    nwl = nc.declare_dram_parameter("nwl", [NV, R], F32, isOutput=False)
    ga_l = nc.declare_dram_parameter(
        "ga_l", [NASLOT, KB // KGRP, 128, KGRP * 512], FP8, isOutput=False)
    ga_r = nc.declare_dram_parameter(
        "ga_r", [NASLOT, KB // KGRP, 128, KGRP * 512], FP8, isOutput=False)
    gb_r = nc.declare_dram_parameter(
        "gb_r", [NBSLOT, KB // KGRP, 128, KGRP * 512], FP8, isOutput=False)

    yo = nc.declare_dram_parameter("yo", [R, NL], F32, isOutput=True)
    zo = nc.declare_dram_parameter("zo", [R, NZ], F32, isOutput=True)
    zso = nc.declare_dram_parameter("zso", [NV, R, NZ], F32, isOutput=True)
    yspo = nc.declare_dram_parameter("yspo", [NV, R, NL], F32, isOutput=True)
    lossp = nc.declare_dram_parameter("lossp", [128, 1], F32, isOutput=True)

    with tile.TileContext(nc) as tc:
        with (
            tc.tile_pool(name="const", bufs=1) as const,
            tc.tile_pool(name="persist", bufs=1) as pp,
            tc.tile_pool(name="small", bufs=2) as sp,
            tc.tile_pool(name="xr", bufs=2) as xr,
            tc.tile_pool(name="h1p", bufs=14) as h1p,
            tc.tile_pool(name="h2p", bufs=13) as h2p,
            tc.tile_pool(name="wp", bufs=3) as wp,
            tc.tile_pool(name="gp", bufs=6) as gp,
            tc.tile_pool(name="gpb", bufs=6) as gpb,
            tc.tile_pool(name="zgp", bufs=4) as zgp,
            tc.tile_pool(name="zgf", bufs=1) as zgf,
            tc.tile_pool(name="ps_enc", bufs=2, space="PSUM") as ps_enc,
            tc.tile_pool(name="ps_a", bufs=4, space="PSUM") as ps_a,
            tc.tile_pool(name="ps_sm", bufs=2, space="PSUM") as ps_sm,
            tc.tile_pool(name="dram", bufs=1, space="DRAM") as dram,
        ):
            # ---------------- constants / small loads ----------------
            ident = const.tile([128, 128], F32)
            make_identity(nc, ident)

            call = const.tile([128, CCOLS], F32, tag="call")
            nc.gpsimd.dma_start(call[:], consts.ap())
            b1c, b2c, bzc, gamc, betc = [], [], [], [], []
            off = 0
            for v in range(NV):
                b1c.append(call[:, off:off + D0P[v] // 128])
                off += D0P[v] // 128
                b2c.append(call[:, off:off + HIDP // 128])
                off += HIDP // 128
                bzc.append(call[:, off:off + 1])
                gamc.append(call[:, off + 1:off + 2])
                betc.append(call[:, off + 2:off + 3])
                off += 3
            bw_t = [call[:, off + s * KB:off + (s + 1) * KB]
                    for s in range(NBSLOT)]
            off += NBSLOT * KB
            cw_t = call[:, off:off + KB]
            off += KB
            regb_t = call[:NL, off:off + 1]
            epsc = call[:, off + 1:off + 2]
            regw_t = const.tile([128, NL], BF16, tag="regw")
            nc.gpsimd.dma_start(regw_t[:], regw.ap())
            nwb_all = const.tile([128, NV, R], F32, tag="nwb")
            nc.gpsimd.dma_start(
                nwb_all[:],
                nwl.ap().rearrange("(o v) f -> o v f", o=1)
                .to_broadcast((128, NV, R)))
            nwb = [nwb_all[:, v, :] for v in range(NV)]
            stats = pp.tile([128, 2 * NV], F32, tag="stats")
            nc.vector.memset(stats[:], 0.0)
            acc = pp.tile([128, 24], F32, tag="acc")
            nc.vector.memset(acc[:], 0.0)
            trash = pp.tile([128, 512], BF16, tag="trash")

            # ---------------- encoders (transposed activations) ---------
            zlT = []
            for v in range(NV if ENC else 0):
                kd = D[v] // 128
                km = D0P[v] // 128
                kh = HIDP // 128
                xtt = xr.tile([128, 16, R], BF16, tag="xt")
                nc.scalar.dma_start(
                    xtt[:, :kd, :], xt[v].ap().rearrange("p (k f) -> p k f", k=kd))
                # h1T = relu(W1^T x^T + b1)
                h1t = []
                for m in range(km):
                    wt = wp.tile([128, 2048], BF16, tag="w")
                    nc.sync.dma_start(wt[:, :D[v]], w1[v].ap()[m])
                    psum = ps_enc.tile([128, R], F32, tag="enc")
                    for k in range(kd):
                        nc.tensor.matmul(psum[:], wt[:, k * 128:(k + 1) * 128],
                                         xtt[:, k, :],
                                         start=(k == 0), stop=(k == kd - 1))
                    h = h1p.tile([128, R], BF16, tag="h1")
                    nc.scalar.activation(h[:], psum[:], AF.Relu,
                                         bias=b1c[v][:, m:m + 1])
                    h1t.append(h)
                # h2T = relu(W2^T h1T + b2)
                h2t = []
                for m in range(kh):
                    wt = wp.tile([128, 2048], BF16, tag="w")
                    nc.sync.dma_start(wt[:, :D0P[v]], w2[v].ap()[m])
                    psum = ps_enc.tile([128, R], F32, tag="enc")
                    for k in range(km):
                        nc.tensor.matmul(psum[:], wt[:, k * 128:(k + 1) * 128],
                                         h1t[k][:],
                                         start=(k == 0), stop=(k == km - 1))
                    h = h2p.tile([128, R], BF16, tag="h2")
                    nc.scalar.activation(h[:], psum[:], AF.Relu,
                                         bias=b2c[v][:, m:m + 1])
                    h2t.append(h)
                # zlT = Wz^T h2T + bz  (fp32, keep; stats fused)
                wt = wp.tile([128, 2048], BF16, tag="w")
                nc.sync.dma_start(wt[:, :HIDP], wz[v].ap()[0])
                psum = ps_enc.tile([128, R], F32, tag="enc")
                for k in range(kh):
                    nc.tensor.matmul(psum[:], wt[:, k * 128:(k + 1) * 128],
                                     h2t[k][:],
                                     start=(k == 0), stop=(k == kh - 1))
                zt = pp.tile([128, R], F32, tag=f"zl{v}")
                ssum = pp.tile([128, 1], F32, tag=f"ssum{v}", name=f"ssum{v}")
                ssq = pp.tile([128, 1], F32, tag=f"ssq{v}", name=f"ssq{v}")
                nc.scalar.activation(zt[:], psum[:], AF.Identity,
                                     bias=bzc[v], accum_out=ssum[:])
                nc.scalar.activation(trash[:], zt[:], AF.Square,
                                     accum_out=ssq[:])
                nc.vector.tensor_copy(stats[:, v:v + 1], ssum[:])
                last_stats_inst = nc.vector.tensor_copy(
                    stats[:, NV + v:NV + v + 1], ssq[:])
                zlT.append(zt)
            if not ENC:
                for v in range(NV):
                    zt = pp.tile([128, R], F32, tag=f"zl{v}", name=f"zl{v}")
                    nc.vector.memset(zt[:], 0.0)
                    zlT.append(zt)

            # ---------------- gram A slots (input-only; overlaps AR) -----
            a_psums = {}

            def a_slot_emit(s, kg_lo, kg_hi):
                if s not in a_psums:
                    a_psums[s] = [
                        ps_a.tile([128, 512], F32, tag="aps", name=f"aps{s}_{j}")
                        for j in range(4)]
                psums = a_psums[s]
                for kg in range(kg_lo, kg_hi):
                    lt = gp.tile([128, KGRP, 512], FP8, tag="gal",
                                 name=f"gal{s}_{kg}")
                    nc.sync.dma_start(
                        lt[:], ga_l.ap()[s, kg].rearrange(
                            "p (k f) -> p k f", k=KGRP))
                    rt = gp.tile([128, KGRP, 512], FP8, tag="gar",
                                 name=f"gar{s}_{kg}")
                    nc.sync.dma_start(
                        rt[:], ga_r.ap()[s, kg].rearrange(
                            "p (k f) -> p k f", k=KGRP))
                    for kk in range(KGRP):
                        first = kg == 0 and kk == 0
                        last = kg == KB // KGRP - 1 and kk == KGRP - 1
                        for j in range(4):
                            nc.tensor.matmul(
                                psums[j][:], lt[:, kk, j * 128:(j + 1) * 128],
                                rt[:, kk, :], start=first, stop=last)
                if kg_hi == KB // KGRP:
                    for j in range(4):
                        nc.scalar.activation(
                            trash[:], psums[j][:], AF.Square,
                            accum_out=acc[:, 4 * s + j:4 * s + j + 1])

            # slot 0 right after the encoders: covers the stats-AllReduce
            # latency; remaining slots are emitted after the BN-dependent
            # PE work so they cover the z-AllGather + B-panel prefetch.
            if GA:
                a_slot_emit(0, 0, KB // KGRP)

            # ---------------- BN stats AllReduce ----------------
            st_in = dram.tile([128, 2 * NV], F32, tag="st_in")
            st_out = dram.tile([128, 2 * NV], F32, tag="st_out",
                               addr_space="Shared")
            statsg = pp.tile([128, 2 * NV], F32, tag="statsg")
            st_dma = None
            if AR:
                st_dma = nc.gpsimd.dma_start(st_in[:], stats[:])
                nc.gpsimd.collective_compute(
                    "AllReduce", ALU.add,
                    ins=[st_in.opt()], outs=[st_out.opt()],
                    replica_groups=[list(range(NCORES))])
                nc.gpsimd.dma_start(statsg[:], st_out[:])
            else:
                nc.vector.tensor_scalar_mul(statsg[:], stats[:], 8.0)

            # ---------------- BN apply + fused z ----------------
            mus = pp.tile([128, NV], F32, tag="mus")
            vart = pp.tile([128, NV], F32, tag="vart")
            nc.vector.tensor_scalar_mul(mus[:], statsg[:, 0:NV], 1.0 / N)
            nc.vector.tensor_scalar_mul(vart[:], statsg[:, NV:2 * NV], 1.0 / N)
            mu2 = pp.tile([128, NV], F32, tag="mu2")
            nc.vector.tensor_tensor(mu2[:], mus[:], mus[:], ALU.mult)
            nc.vector.tensor_tensor(vart[:], vart[:], mu2[:], ALU.subtract)
            stdt = pp.tile([128, NV], F32, tag="stdt")
            nc.scalar.activation(stdt[:], vart[:], AF.Sqrt, bias=epsc)
            invstd = pp.tile([128, NV], F32, tag="invstd")
            nc.vector.reciprocal(invstd[:], stdt[:])
            gall = pp.tile([128, NV], F32, tag="gall")
            ball = pp.tile([128, NV], F32, tag="ball")
            for v in range(NV):
                nc.vector.tensor_copy(gall[:, v:v + 1], gamc[v])
                nc.vector.tensor_copy(ball[:, v:v + 1], betc[v])
            scl = pp.tile([128, NV], F32, tag="scl")
            nc.vector.tensor_tensor(scl[:], gall[:], invstd[:], ALU.mult)
            shf = pp.tile([128, NV], F32, tag="shf")
            nc.vector.tensor_tensor(shf[:], mus[:], scl[:], ALU.mult)
            nc.vector.tensor_tensor(shf[:], ball[:], shf[:], ALU.subtract)

            ziT = []
            for v in range(NV):
                zi = pp.tile([128, R], F32, tag=f"zi{v}")
                nc.scalar.activation(zi[:], zlT[v][:], AF.Identity,
                                     bias=shf[:, v:v + 1], scale=scl[:, v:v + 1])
                ziT.append(zi)
            zT = pp.tile([128, R], F32, tag="zT")
            ztmp = pp.tile([128, R], F32, tag="ztmp")
            nc.vector.tensor_tensor(zT[:], ziT[0][:], nwb[0], ALU.mult)
            for v in range(1, NV):
                nc.vector.tensor_tensor(ztmp[:], ziT[v][:], nwb[v], ALU.mult)
                nc.vector.tensor_tensor(zT[:], zT[:], ztmp[:], ALU.add)

            # ---------------- natural-layout outputs + z AllGather -------
            zag_in = dram.tile([R, NZ], F32, tag="zag_in")
            zag_out = dram.tile([N, NZ], F32, tag="zag_out", addr_space="Shared")
            znat = sp.tile([128, RB, 128], F32, tag="znat", name="znat")
            for j in range(RB):
                pst = ps_sm.tile([128, 128], F32, tag="sm", name=f"ztp{j}")
                nc.tensor.transpose(pst[:], zT[:, j * 128:(j + 1) * 128], ident[:])
                nc.vector.tensor_copy(znat[:, j, :], pst[:])
            nc.gpsimd.dma_start(zo.ap().rearrange("(j p) f -> p j f", p=128), znat[:])
            nc.sync.dma_start(zag_in.rearrange("(j p) f -> p j f", p=128), znat[:])
            if AG:
                nc.gpsimd.collective_compute(
                    "AllGather", ALU.bypass,
                    ins=[zag_in.opt()], outs=[zag_out.opt()],
                    replica_groups=[list(range(NCORES))])
            for v in range(NV):
                zsnat = sp.tile([128, RB, 128], F32, tag="zsnat", name=f"zsnat{v}")
                for j in range(RB):
                    pst = ps_sm.tile([128, 128], F32, tag="sm", name=f"zstp{v}_{j}")
                    nc.tensor.transpose(pst[:], ziT[v][:, j * 128:(j + 1) * 128],
                                        ident[:])
                    nc.vector.tensor_copy(zsnat[:, j, :], pst[:])
                nc.gpsimd.dma_start(
                    zso.ap()[v].rearrange("(j p) f -> p j f", p=128), zsnat[:])

            # ---------------- reg heads (sigmoid(relu(z) W + b)) ---------
            def reg_head(zin, out_ap, label):
                rz = sp.tile([128, R], BF16, tag="reluz", name=f"rz_{label}")
                nc.scalar.activation(rz[:], zin[:], AF.Relu)
                psum = ps_sm.tile([NL, R], F32, tag="sm", name=f"regp_{label}")
                nc.tensor.matmul(psum[:], regw_t[:], rz[:], start=True, stop=True)
                yt = sp.tile([NL, R], F32, tag="yT", name=f"yt_{label}")
                nc.scalar.activation(yt[:], psum[:], AF.Sigmoid,
                                     bias=regb_t)
                ynat = sp.tile([128, RB, NL], F32, tag="ynat", name=f"yn_{label}")
                for j in range(RB):
                    pst = ps_sm.tile([128, NL], F32, tag="sm",
                                     name=f"ytp_{label}_{j}")
                    nc.tensor.matmul(pst[:], yt[:, j * 128:(j + 1) * 128],
                                     ident[:NL, :NL], is_transpose=True,
                                     start=True, stop=True)
                    nc.vector.tensor_copy(ynat[:, j, :], pst[:])
                nc.gpsimd.dma_start(
                    out_ap.rearrange("(j p) f -> p j f", p=128), ynat[:])

            reg_head(zT, yo.ap(), "y")
            for v in range(NV):
                reg_head(ziT[v], yspo.ap()[v], f"ysp{v}")

            if GA:
                for s in range(1, NASLOT):
                    a_slot_emit(s, 0, KB // KGRP)

            # ---------------- gathered Z: rz, scaled tiles, B & C --------
            rz2 = pp.tile([128, KB], F32, tag="rz2")
            zgt = zgf.tile([128, KB, 128], F32, tag="zg")
            if BC:
                nc.gpsimd.dma_start(
                    zgt[:], zag_out.rearrange("(k p) f -> p k f", p=128))
                for kb in range(KB):
                    nc.scalar.activation(trash[:, :128], zgt[:, kb, :], AF.Square,
                                         accum_out=rz2[:, kb:kb + 1])
            else:
                nc.vector.memset(rz2[:], 1.0)
            rzt = pp.tile([128, KB], F32, tag="rzt")
            nc.scalar.activation(rzt[:], rz2[:], AF.Sqrt)
            nc.vector.tensor_scalar_max(rzt[:], rzt[:], 1e-12)
            rzinv = pp.tile([128, KB], F32, tag="rzinv")
            nc.vector.reciprocal(rzinv[:], rzt[:])

            combB = [pp.tile([128, KB], F32, tag=f"combB{s}", name=f"combB{s}")
                     for s in range(NBSLOT)]
            for s in range(NBSLOT):
                nc.vector.tensor_tensor(combB[s][:], bw_t[s], rzinv[:],
                                        ALU.mult)
            combC = pp.tile([128, KB], F32, tag="combC")
            nc.vector.tensor_tensor(combC[:], cw_t, rzinv[:], ALU.mult)

            # B slots: psum [NZ, 512] = sum_kb (D_v Z)[kb]^T @ (D_u X)[kb, chunk]
            for s in range(NBSLOT if BC else 0):
                psum = ps_a.tile([128, 512], F32, tag="aps", name=f"bps{s}")
                for kg in range(KB // KGRP):
                    rt = gpb.tile([128, KGRP, 512], FP8, tag="gbr")
                    nc.scalar.dma_start(
                        rt[:], gb_r.ap()[s, kg].rearrange(
                            "p (k f) -> p k f", k=KGRP))
                    for kk in range(KGRP):
                        kb = kg * KGRP + kk
                        sz = zgp.tile([128, 128], FP8, tag="szb")
                        nc.vector.tensor_scalar_mul(sz[:], zgt[:, kb, :],
                                                    combB[s][:, kb:kb + 1])
                        nc.tensor.matmul(psum[:], sz[:], rt[:, kk, :],
                                         start=(kb == 0), stop=(kb == KB - 1))
                nc.scalar.activation(trash[:], psum[:], AF.Square,
                                     accum_out=acc[:, 16 + s:17 + s])

            # C: psum [NZ, NZ] = sum_kb (D_v Z)[kb]^T @ (D_v Z)[kb]
            if BC:
                psum = ps_sm.tile([128, 128], F32, tag="sm", name="cps")
                for kb in range(KB):
                    sc = zgp.tile([128, 128], BF16, tag="szc")
                    nc.vector.tensor_scalar_mul(sc[:], zgt[:, kb, :],
                                                combC[:, kb:kb + 1])
                    nc.tensor.matmul(psum[:], sc[:], sc[:],
                                     start=(kb == 0), stop=(kb == KB - 1))
                nc.scalar.activation(trash[:, :128], psum[:], AF.Square,
                                     accum_out=acc[:, 18:19])

            # ---------------- loss partial ----------------
            ra = pp.tile([128, 1], F32, tag="ra")
            rb = pp.tile([128, 1], F32, tag="rb")
            nc.vector.tensor_reduce(ra[:], acc[:, 0:16], mybir.AxisListType.X,
                                    ALU.add)
            nc.vector.tensor_scalar_mul(ra[:], ra[:], float(FP8_SCALE ** -4))
            nc.vector.tensor_reduce(rb[:], acc[:, 16:18], mybir.AxisListType.X,
                                    ALU.add)
            nc.vector.tensor_scalar_mul(
                rb[:], rb[:], float(-2.0 * (FP8_SCALE * SZ_SCALE) ** -2))
            nc.vector.tensor_tensor(ra[:], ra[:], rb[:], ALU.add)
            nc.vector.tensor_tensor(ra[:], ra[:], acc[:, 18:19], ALU.add)
            lt = pp.tile([128, 1], F32, tag="lossf")
            nc.vector.tensor_copy(lt[:], ra[:])
            nc.sync.dma_start(lossp.ap()[:, :], lt[:])

    nc.compile()
    return nc


_NC = None


def _get_nc():
    global _NC
    if _NC is None:
        _NC = build_nc()
    return _NC


def _prep_in_maps(inputs):
    bf = ml_dtypes.bfloat16
    f8 = mybir.dt.np(FP8)
    xs = [np.ascontiguousarray(np.asarray(inputs[f"x{i}"], dtype=np.float32))
          for i in range(NV)]
    we = np.asarray(inputs["we"], dtype=np.float32)
    p = inputs["params"]
    enc = p["enc"]
    var = np.asarray(p["variables"], dtype=np.float32)

    we_s = we * var[None, :]
    nw = we_s / we_s.sum(axis=1, keepdims=True)
    u = []
    for i in range(NV):
        r = np.sqrt((xs[i] ** 2).sum(axis=1))
        u.append(we_s[:, i] / np.maximum(r, 1e-12))

    def swz(w, kp, mp):
        # -> [mp//128, 128, kp]: out[m, p, k*128+f] = w_pad[k*128+p, m*128+f]
        k, m = w.shape
        out = np.zeros((kp, mp), np.float32)
        out[:k, :m] = np.asarray(w, np.float32)
        out = out.reshape(kp // 128, 128, mp // 128, 128)
        out = out.transpose(2, 1, 0, 3).reshape(mp // 128, 128, kp)
        return np.ascontiguousarray(out).astype(bf)

    def padv(b, n):
        out = np.zeros((n,), np.float32)
        out[: b.shape[0]] = np.asarray(b, np.float32)
        return out

    shared = {}
    for v in range(NV):
        e = enc[v]
        shared[f"w1_{v}"] = swz(e["W1"], D[v], D0P[v])
        shared[f"w2_{v}"] = swz(e["W2"], D0P[v], HIDP)
        shared[f"wz_{v}"] = swz(e["Wz"], HIDP, 128)
    shared["regw"] = np.asarray(p["reg_W"], np.float32).astype(bf)

    CCOLS = sum(D0P[v] // 128 + HIDP // 128 + 3 for v in range(NV)) + 3 * KB + 2
    # bw/cw columns are per-core; consts base (weights/biases) is shared
    cbase = np.zeros((128, CCOLS), np.float32)
    off = 0
    for v in range(NV):
        e = enc[v]
        nb1 = D0P[v] // 128
        cbase[:, off:off + nb1] = padv(e["b1"], D0P[v]).reshape(nb1, 128).T
        off += nb1
        nb2 = HIDP // 128
        cbase[:, off:off + nb2] = padv(e["b2"], HIDP).reshape(nb2, 128).T
        off += nb2
        cbase[:, off] = padv(e["bz"], NZ)
        cbase[:, off + 1] = np.asarray(e["gamma"], np.float32)
        cbase[:, off + 2] = np.asarray(e["beta"], np.float32)
        off += 3
    bw_off = off
    cw_off = off + NBSLOT * KB
    regb_off = cw_off + KB
    cbase[:NL, regb_off] = np.asarray(p["reg_b"], np.float32)
    cbase[:, regb_off + 1] = BN_EPS

    sqrt2 = np.float32(math.sqrt(2.0))
    in_maps = []
    for c in range(NCORES):
        rows = slice(c * R, (c + 1) * R)
        m = dict(shared)
        for v in range(NV):
            kd = D[v] // 128
            xtv = xs[v][rows].T.reshape(kd, 128, R).transpose(1, 0, 2)
            m[f"xt{v}"] = np.ascontiguousarray(
                xtv.reshape(128, kd * R)).astype(bf)
        m["nwl"] = np.ascontiguousarray(nw[rows].T)

        gal = np.zeros((NASLOT, N, 512), f8)
        gar = np.zeros((NASLOT, N, 512), f8)
        KGG = KB // KGRP
        for s in range(NASLOT):
            idx = c * NASLOT + s
            if idx >= len(A_PIECES):
                continue
            v, cs, w, ms = A_PIECES[idx]
            uw = u[v][:, None] * np.float32(FP8_SCALE)
            fac = sqrt2 if ms < cs else np.float32(1.0)
            lw = min(512, D[v] - ms)
            gal[s, :, :lw] = (xs[v][:, ms:ms + lw] * (uw * fac)).astype(f8)
            gar[s, :, :w] = (xs[v][:, cs:cs + w] * uw).astype(f8)
        def gsw(g, nslot):
            # [S, N, 512] -> [S, KGG, 128, KGRP*512]
            g = g.reshape(nslot, KGG, KGRP, 128, 512).transpose(0, 1, 3, 2, 4)
            return np.ascontiguousarray(
                g.reshape(nslot, KGG, 128, KGRP * 512))
        m["ga_l"] = gsw(gal, NASLOT)
        m["ga_r"] = gsw(gar, NASLOT)

        gbr = np.zeros((NBSLOT, N, 512), f8)
        cc = cbase.copy()
        for s in range(NBSLOT):
            v, cs, w = B_UNITS[c * NBSLOT + s]
            gbr[s, :, :w] = (xs[v][:, cs:cs + w]
                             * (u[v][:, None] * np.float32(FP8_SCALE))).astype(f8)
            cc[:, bw_off + s * KB:bw_off + (s + 1) * KB] = \
                we_s[:, v].reshape(KB, 128).T * np.float32(SZ_SCALE)
        m["gb_r"] = gsw(gbr, NBSLOT)
        if c < NV:
            cc[:, cw_off:cw_off + KB] = we_s[:, c].reshape(KB, 128).T
        m["consts"] = cc
        in_maps.append(m)
    return in_maps


def kernel(**inputs):
    nc = _get_nc()
    in_maps = _prep_in_maps(inputs)
    res = run_bass_kernel_spmd(nc, in_maps, core_ids=list(range(NCORES)))
    r = res.results
    yL = np.concatenate([r[c]["yo"] for c in range(NCORES)], axis=0)
    z = np.concatenate([r[c]["zo"] for c in range(NCORES)], axis=0)
    zs = np.concatenate([r[c]["zso"] for c in range(NCORES)], axis=1)
    ysp = np.concatenate([r[c]["yspo"] for c in range(NCORES)], axis=1)
    total = np.float64(0.0)
    for c in range(NCORES):
        total += np.float64(r[c]["lossp"].sum())
    loss = np.float32(total / (float(N) * float(N)) / 6.0)
    return (yL, z, zs, ysp, loss)


# revision 28
# speedup vs baseline: 1.1442x; 1.0638x over previous
"""Trainium2 Bass kernel for nn_AE_89343909691943 (multi-view AE loss_fn).

8-core SPMD strategy (data-parallel over the N=4096 sample axis, 512 rows/core):
  * 6 per-view encoder MLPs run row-sharded in transposed-activation layout
    (features on partitions), so bias+ReLU fuse into PSUM evacuation and
    BatchNorm stats are per-partition reductions.
  * BN batch stats via a tiny AllReduce; fused z via an AllGather of z.
  * The N x N similarity matrices are never materialized.  The loss
      loss_i = mean((w (sims_i - sim2) w)^2)
    is computed exactly as
      (||X^T D_{u^2} X||_F^2 - 2||X^T D_{uv} Z||_F^2 + ||Z^T D_{v^2} Z||_F^2)/N^2
    with u = w/||x_row||, v = w/||z_row||, which shrinks the gram work from
    O(N^2 d) to O(N d^2) and keeps everything on the TensorEngine.
  * All GEMMs run in bf16 with fp32 PSUM accumulation (validated ~0.5% worst
    output error, loss error ~1e-5).
  * DMAs are batched (one instruction per weight m-panel / k-group) to keep
    the Sync sequencer's descriptor generation off the critical path.
"""

import sys

for _p in ("/opt/trn_rl_repo", "/root/.axon_site/_ro/trn_rl_repo"):
    if _p not in sys.path:
        sys.path.insert(0, _p)

import math

import ml_dtypes
import numpy as np

import concourse.bass as bass
import concourse.mybir as mybir
import concourse.tile as tile
from concourse import bacc
from concourse.bass_utils import run_bass_kernel_spmd
from concourse.masks import make_identity

BF16 = mybir.dt.bfloat16
FP8 = mybir.dt.float8e4
FP8_SCALE = 64.0
SZ_SCALE = 16.0
F32 = mybir.dt.float32
AF = mybir.ActivationFunctionType
ALU = mybir.AluOpType

N = 4096
NCORES = 8
R = N // NCORES  # 512 rows per core
NZ = 128
NL = 80
NV = 6
D = [1024, 1536, 2048, 1280, 896, 1024]
D0 = [819, 1229, 1638, 1024, 717, 819]
D0P = [896, 1280, 1664, 1024, 768, 896]
HID = 1500
HIDP = 1536
BN_EPS = 1e-5
KB = N // 128  # 32 row-blocks of 128
RB = R // 128  # 4 local row-blocks
KGRP = 4  # gram k-blocks fetched per DMA

NASLOT = 4  # gram-A pieces per core (uniform across cores)
NBSLOT = 2  # gram-B (cross term) units per core


def _a_pieces():
    """(view, chunk_start, chunk_width, m_start) gram-A pieces.

    A_i = (D_u X_i)^T (D_u X_i) is computed in 512-wide column chunks; for
    each chunk only m <= chunk_end 512-wide lhsT pieces are needed (symmetric
    matrix; strictly-below pieces get a sqrt(2) factor folded into the host
    scaling so ||.||^2 counts them twice)."""
    pieces = []
    for v in range(NV):
        for cs in range(0, D[v], 512):
            w = min(512, D[v] - cs)
            for ms in range(0, cs + 1, 512):
                pieces.append((v, cs, w, ms))
    return pieces


def _b_units():
    units = []
    for v in range(NV):
        for cs in range(0, D[v], 512):
            units.append((v, cs, min(512, D[v] - cs)))
    return units


A_PIECES = _a_pieces()  # 31 pieces -> 32 slots (one zero slot)
B_UNITS = _b_units()  # 16 units -> exactly 2 per core
assert len(A_PIECES) <= NASLOT * NCORES
assert len(B_UNITS) == NBSLOT * NCORES


def build_nc(stages=frozenset({"enc", "ar", "a", "ag", "bc"})):
    nc = bacc.Bacc("TRN2", target_bir_lowering=False, debug=False,
                   num_devices=NCORES)

    ENC = "enc" in stages
    AR = "ar" in stages
    GA = "a" in stages
    AG = "ag" in stages
    BC = "bc" in stages

    # ---------------- parameters ----------------
    # weights are host-swizzled to [m_tile, 128, K*128]:
    #   w[m, p, k*128+f] = W_padded[k*128 + p, m*128 + f]
    xt = [nc.declare_dram_parameter(f"xt{v}", [128, D[v] // 128 * R], BF16,
                                    isOutput=False)
          for v in range(NV)]
    w1 = [nc.declare_dram_parameter(
        f"w1_{v}", [D0P[v] // 128, 128, D[v]], BF16, isOutput=False)
        for v in range(NV)]
    w2 = [nc.declare_dram_parameter(
        f"w2_{v}", [HIDP // 128, 128, D0P[v]], BF16, isOutput=False)
        for v in range(NV)]
    wz = [nc.declare_dram_parameter(
        f"wz_{v}", [1, 128, HIDP], BF16, isOutput=False)
        for v in range(NV)]
    # all small per-partition constants packed into one [128, CCOLS] param:
    # cols: per view [b1c(D0P/128) b2c(12) bz gam bet] then bw(2*32) cw(32)
    # regb(1) eps(1)
    CCOLS = sum(D0P[v] // 128 + HIDP // 128 + 3 for v in range(NV)) + 3 * KB + 2
    consts = nc.declare_dram_parameter("consts", [128, CCOLS], F32,
                                       isOutput=False)
    regw = nc.declare_dram_parameter("regw", [NZ, NL], BF16, isOutput=False)
    nwl = nc.declare_dram_parameter("nwl", [NV, R], F32, isOutput=False)
    ga_l = nc.declare_dram_parameter(
        "ga_l", [NASLOT, KB // KGRP, 128, KGRP * 512], FP8, isOutput=False)
    ga_r = nc.declare_dram_parameter(
        "ga_r", [NASLOT, KB // KGRP, 128, KGRP * 512], FP8, isOutput=False)
    gb_r = nc.declare_dram_parameter(
        "gb_r", [NBSLOT, KB // KGRP, 128, KGRP * 512], FP8, isOutput=False)

    yo = nc.declare_dram_parameter("yo", [R, NL], F32, isOutput=True)
    zo = nc.declare_dram_parameter("zo", [R, NZ], F32, isOutput=True)
    zso = nc.declare_dram_parameter("zso", [NV, R, NZ], F32, isOutput=True)
    yspo = nc.declare_dram_parameter("yspo", [NV, R, NL], F32, isOutput=True)
    lossp = nc.declare_dram_parameter("lossp", [128, 1], F32, isOutput=True)

    with tile.TileContext(nc) as tc:
        with (
            tc.tile_pool(name="const", bufs=1) as const,
            tc.tile_pool(name="persist", bufs=1) as pp,
            tc.tile_pool(name="small", bufs=2) as sp,
            tc.tile_pool(name="xr", bufs=2) as xr,
            tc.tile_pool(name="h1p", bufs=14) as h1p,
            tc.tile_pool(name="h2p", bufs=13) as h2p,
            tc.tile_pool(name="wp", bufs=3) as wp,
            tc.tile_pool(name="gp", bufs=6) as gp,
            tc.tile_pool(name="gpb", bufs=6) as gpb,
            tc.tile_pool(name="zgp", bufs=4) as zgp,
            tc.tile_pool(name="zgf", bufs=1) as zgf,
            tc.tile_pool(name="ps_enc", bufs=2, space="PSUM") as ps_enc,
            tc.tile_pool(name="ps_a", bufs=4, space="PSUM") as ps_a,
            tc.tile_pool(name="ps_sm", bufs=2, space="PSUM") as ps_sm,
            tc.tile_pool(name="dram", bufs=1, space="DRAM") as dram,
        ):
            # ---------------- constants / small loads ----------------
            ident = const.tile([128, 128], F32)
            make_identity(nc, ident)

            call = const.tile([128, CCOLS], F32, tag="call")
            nc.gpsimd.dma_start(call[:], consts.ap())
            b1c, b2c, bzc, gamc, betc = [], [], [], [], []
            off = 0
            for v in range(NV):
                b1c.append(call[:, off:off + D0P[v] // 128])
                off += D0P[v] // 128
                b2c.append(call[:, off:off + HIDP // 128])
                off += HIDP // 128
                bzc.append(call[:, off:off + 1])
                gamc.append(call[:, off + 1:off + 2])
                betc.append(call[:, off + 2:off + 3])
                off += 3
            bw_t = [call[:, off + s * KB:off + (s + 1) * KB]
                    for s in range(NBSLOT)]
            off += NBSLOT * KB
            cw_t = call[:, off:off + KB]
            off += KB
            regb_t = call[:NL, off:off + 1]
            epsc = call[:, off + 1:off + 2]
            regw_t = const.tile([128, NL], BF16, tag="regw")
            nwb_all = const.tile([128, NV, R], F32, tag="nwb")
            nwb = [nwb_all[:, v, :] for v in range(NV)]
            stats = pp.tile([128, 2 * NV], F32, tag="stats")
            nc.vector.memset(stats[:], 0.0)
            acc = pp.tile([128, 24], F32, tag="acc")
            nc.vector.memset(acc[:], 0.0)
            trash = pp.tile([128, 512], BF16, tag="trash")

            # ---------------- encoders (transposed activations) ---------
            zlT = []
            for v in range(NV if ENC else 0):
                kd = D[v] // 128
                km = D0P[v] // 128
                kh = HIDP // 128
                xtt = xr.tile([128, 16, R], BF16, tag="xt")
                nc.scalar.dma_start(
                    xtt[:, :kd, :], xt[v].ap().rearrange("p (k f) -> p k f", k=kd))
                # h1T = relu(W1^T x^T + b1)
                h1t = []
                for m in range(km):
                    wt = wp.tile([128, 2048], BF16, tag="w")
                    nc.sync.dma_start(wt[:, :D[v]], w1[v].ap()[m])
                    psum = ps_enc.tile([128, R], F32, tag="enc")
                    for k in range(kd):
                        nc.tensor.matmul(psum[:], wt[:, k * 128:(k + 1) * 128],
                                         xtt[:, k, :],
                                         start=(k == 0), stop=(k == kd - 1))
                    h = h1p.tile([128, R], BF16, tag="h1")
                    nc.scalar.activation(h[:], psum[:], AF.Relu,
                                         bias=b1c[v][:, m:m + 1])
                    h1t.append(h)
                # h2T = relu(W2^T h1T + b2)
                h2t = []
                for m in range(kh):
                    wt = wp.tile([128, 2048], BF16, tag="w")
                    nc.sync.dma_start(wt[:, :D0P[v]], w2[v].ap()[m])
                    psum = ps_enc.tile([128, R], F32, tag="enc")
                    for k in range(km):
                        nc.tensor.matmul(psum[:], wt[:, k * 128:(k + 1) * 128],
                                         h1t[k][:],
                                         start=(k == 0), stop=(k == km - 1))
                    h = h2p.tile([128, R], BF16, tag="h2")
                    nc.scalar.activation(h[:], psum[:], AF.Relu,
                                         bias=b2c[v][:, m:m + 1])
                    h2t.append(h)
                # zlT = Wz^T h2T + bz  (fp32, keep; stats fused)
                wt = wp.tile([128, 2048], BF16, tag="w")
                nc.sync.dma_start(wt[:, :HIDP], wz[v].ap()[0])
                psum = ps_enc.tile([128, R], F32, tag="enc")
                for k in range(kh):
                    nc.tensor.matmul(psum[:], wt[:, k * 128:(k + 1) * 128],
                                     h2t[k][:],
                                     start=(k == 0), stop=(k == kh - 1))
                zt = pp.tile([128, R], F32, tag=f"zl{v}")
                ssum = pp.tile([128, 1], F32, tag=f"ssum{v}", name=f"ssum{v}")
                ssq = pp.tile([128, 1], F32, tag=f"ssq{v}", name=f"ssq{v}")
                nc.scalar.activation(zt[:], psum[:], AF.Identity,
                                     bias=bzc[v], accum_out=ssum[:])
                nc.scalar.activation(trash[:], zt[:], AF.Square,
                                     accum_out=ssq[:])
                nc.vector.tensor_copy(stats[:, v:v + 1], ssum[:])
                last_stats_inst = nc.vector.tensor_copy(
                    stats[:, NV + v:NV + v + 1], ssq[:])
                zlT.append(zt)
            if not ENC:
                for v in range(NV):
                    zt = pp.tile([128, R], F32, tag=f"zl{v}", name=f"zl{v}")
                    nc.vector.memset(zt[:], 0.0)
                    zlT.append(zt)

            # ---------------- gram A slots (input-only; overlaps AR) -----
            a_psums = {}

            def a_slot_emit(s, kg_lo, kg_hi):
                if s not in a_psums:
                    a_psums[s] = [
                        ps_a.tile([128, 512], F32, tag="aps", name=f"aps{s}_{j}")
                        for j in range(4)]
                psums = a_psums[s]
                for kg in range(kg_lo, kg_hi):
                    lt = gp.tile([128, KGRP, 512], FP8, tag="gal",
                                 name=f"gal{s}_{kg}")
                    nc.sync.dma_start(
                        lt[:], ga_l.ap()[s, kg].rearrange(
                            "p (k f) -> p k f", k=KGRP))
                    rt = gp.tile([128, KGRP, 512], FP8, tag="gar",
                                 name=f"gar{s}_{kg}")
                    nc.sync.dma_start(
                        rt[:], ga_r.ap()[s, kg].rearrange(
                            "p (k f) -> p k f", k=KGRP))
                    for kk in range(KGRP):
                        first = kg == 0 and kk == 0
                        last = kg == KB // KGRP - 1 and kk == KGRP - 1
                        for j in range(4):
                            nc.tensor.matmul(
                                psums[j][:], lt[:, kk, j * 128:(j + 1) * 128],
                                rt[:, kk, :], start=first, stop=last)
                if kg_hi == KB // KGRP:
                    for j in range(4):
                        nc.scalar.activation(
                            trash[:], psums[j][:], AF.Square,
                            accum_out=acc[:, 4 * s + j:4 * s + j + 1])

            # slot 0 right after the encoders: covers the stats-AllReduce
            # latency; remaining slots are emitted after the BN-dependent
            # PE work so they cover the z-AllGather + B-panel prefetch.
            if GA:
                a_slot_emit(0, 0, KB // KGRP)

            # ---------------- BN stats AllReduce ----------------
            st_in = dram.tile([128, 2 * NV], F32, tag="st_in")
            st_out = dram.tile([128, 2 * NV], F32, tag="st_out",
                               addr_space="Shared")
            statsg = pp.tile([128, 2 * NV], F32, tag="statsg")
            st_dma = None
            if AR:
                st_dma = nc.gpsimd.dma_start(st_in[:], stats[:])
                nc.gpsimd.collective_compute(
                    "AllReduce", ALU.add,
                    ins=[st_in.opt()], outs=[st_out.opt()],
                    replica_groups=[list(range(NCORES))])
                nc.gpsimd.dma_start(statsg[:], st_out[:])
            else:
                nc.vector.tensor_scalar_mul(statsg[:], stats[:], 8.0)

            if GA:
                a_slot_emit(1, 0, KB // KGRP)

            # ---------------- BN apply + fused z ----------------
            mus = pp.tile([128, NV], F32, tag="mus")
            vart = pp.tile([128, NV], F32, tag="vart")
            nc.vector.tensor_scalar_mul(mus[:], statsg[:, 0:NV], 1.0 / N)
            nc.vector.tensor_scalar_mul(vart[:], statsg[:, NV:2 * NV], 1.0 / N)
            mu2 = pp.tile([128, NV], F32, tag="mu2")
            nc.vector.tensor_tensor(mu2[:], mus[:], mus[:], ALU.mult)
            nc.vector.tensor_tensor(vart[:], vart[:], mu2[:], ALU.subtract)
            stdt = pp.tile([128, NV], F32, tag="stdt")
            nc.scalar.activation(stdt[:], vart[:], AF.Sqrt, bias=epsc)
            invstd = pp.tile([128, NV], F32, tag="invstd")
            nc.vector.reciprocal(invstd[:], stdt[:])
            gall = pp.tile([128, NV], F32, tag="gall")
            ball = pp.tile([128, NV], F32, tag="ball")
            for v in range(NV):
                nc.vector.tensor_copy(gall[:, v:v + 1], gamc[v])
                nc.vector.tensor_copy(ball[:, v:v + 1], betc[v])
            scl = pp.tile([128, NV], F32, tag="scl")
            nc.vector.tensor_tensor(scl[:], gall[:], invstd[:], ALU.mult)
            shf = pp.tile([128, NV], F32, tag="shf")
            nc.vector.tensor_tensor(shf[:], mus[:], scl[:], ALU.mult)
            nc.vector.tensor_tensor(shf[:], ball[:], shf[:], ALU.subtract)

            nc.scalar.dma_start(regw_t[:], regw.ap())
            nc.scalar.dma_start(
                nwb_all[:],
                nwl.ap().rearrange("(o v) f -> o v f", o=1)
                .to_broadcast((128, NV, R)))
            ziT = []
            for v in range(NV):
                zi = pp.tile([128, R], F32, tag=f"zi{v}")
                nc.scalar.activation(zi[:], zlT[v][:], AF.Identity,
                                     bias=shf[:, v:v + 1], scale=scl[:, v:v + 1])
                ziT.append(zi)
            zT = pp.tile([128, R], F32, tag="zT")
            ztmp = pp.tile([128, R], F32, tag="ztmp")
            nc.vector.tensor_tensor(zT[:], ziT[0][:], nwb[0], ALU.mult)
            for v in range(1, NV):
                nc.vector.tensor_tensor(ztmp[:], ziT[v][:], nwb[v], ALU.mult)
                nc.vector.tensor_tensor(zT[:], zT[:], ztmp[:], ALU.add)

            # ---------------- natural-layout outputs + z AllGather -------
            ZW = NZ + 4  # z columns + rz^2 + pad
            zag_in = dram.tile([R, ZW], F32, tag="zag_in")
            zag_out = dram.tile([N, ZW], F32, tag="zag_out", addr_space="Shared")
            znat = sp.tile([128, RB, ZW], F32, tag="znat", name="znat")
            for j in range(RB):
                pst = ps_sm.tile([128, 128], F32, tag="sm", name=f"ztp{j}")
                nc.tensor.transpose(pst[:], zT[:, j * 128:(j + 1) * 128], ident[:])
                nc.vector.tensor_copy(znat[:, j, :NZ], pst[:])
                nc.scalar.activation(trash[:, :128], znat[:, j, :NZ], AF.Square,
                                     accum_out=znat[:, j, NZ:NZ + 1])
            nc.gpsimd.dma_start(zo.ap().rearrange("(j p) f -> p j f", p=128),
                                znat[:, :, :NZ])
            nc.sync.dma_start(zag_in.rearrange("(j p) f -> p j f", p=128), znat[:])
            if AG:
                nc.gpsimd.collective_compute(
                    "AllGather", ALU.bypass,
                    ins=[zag_in.opt()], outs=[zag_out.opt()],
                    replica_groups=[list(range(NCORES))])
            for v in range(NV):
                zsnat = sp.tile([128, RB, 128], F32, tag="zsnat", name=f"zsnat{v}")
                for j in range(RB):
                    pst = ps_sm.tile([128, 128], F32, tag="sm", name=f"zstp{v}_{j}")
                    nc.tensor.transpose(pst[:], ziT[v][:, j * 128:(j + 1) * 128],
                                        ident[:])
                    nc.vector.tensor_copy(zsnat[:, j, :], pst[:])
                nc.gpsimd.dma_start(
                    zso.ap()[v].rearrange("(j p) f -> p j f", p=128), zsnat[:])

            # ---------------- reg heads (sigmoid(relu(z) W + b)) ---------
            def reg_head(zin, out_ap, label):
                rz = sp.tile([128, R], BF16, tag="reluz", name=f"rz_{label}")
                nc.scalar.activation(rz[:], zin[:], AF.Relu)
                psum = ps_sm.tile([NL, R], F32, tag="sm", name=f"regp_{label}")
                nc.tensor.matmul(psum[:], regw_t[:], rz[:], start=True, stop=True)
                yt = sp.tile([NL, R], F32, tag="yT", name=f"yt_{label}")
                nc.scalar.activation(yt[:], psum[:], AF.Sigmoid,
                                     bias=regb_t)
                ynat = sp.tile([128, RB, NL], F32, tag="ynat", name=f"yn_{label}")
                for j in range(RB):
                    pst = ps_sm.tile([128, NL], F32, tag="sm",
                                     name=f"ytp_{label}_{j}")
                    nc.tensor.matmul(pst[:], yt[:, j * 128:(j + 1) * 128],
                                     ident[:NL, :NL], is_transpose=True,
                                     start=True, stop=True)
                    nc.vector.tensor_copy(ynat[:, j, :], pst[:])
                nc.gpsimd.dma_start(
                    out_ap.rearrange("(j p) f -> p j f", p=128), ynat[:])

            reg_head(zT, yo.ap(), "y")
            for v in range(NV):
                reg_head(ziT[v], yspo.ap()[v], f"ysp{v}")

            if GA:
                for s in range(2, NASLOT):
                    a_slot_emit(s, 0, KB // KGRP)

            # ---------------- gathered Z: rz, scaled tiles, B & C --------
            rz2 = pp.tile([128, KB], F32, tag="rz2")
            zgt = zgf.tile([128, KB, ZW], F32, tag="zg")
            if BC:
                nc.gpsimd.dma_start(
                    zgt[:], zag_out.rearrange("(k p) f -> p k f", p=128))
                nc.vector.tensor_copy(rz2[:], zgt[:, :, NZ])
            else:
                nc.vector.memset(rz2[:], 1.0)
            rzt = pp.tile([128, KB], F32, tag="rzt")
            nc.scalar.activation(rzt[:], rz2[:], AF.Sqrt)
            nc.vector.tensor_scalar_max(rzt[:], rzt[:], 1e-12)
            rzinv = pp.tile([128, KB], F32, tag="rzinv")
            nc.vector.reciprocal(rzinv[:], rzt[:])

            combB = [pp.tile([128, KB], F32, tag=f"combB{s}", name=f"combB{s}")
                     for s in range(NBSLOT)]
            for s in range(NBSLOT):
                nc.vector.tensor_tensor(combB[s][:], bw_t[s], rzinv[:],
                                        ALU.mult)
            combC = pp.tile([128, KB], F32, tag="combC")
            nc.vector.tensor_tensor(combC[:], cw_t, rzinv[:], ALU.mult)

            # B slots: psum [NZ, 512] = sum_kb (D_v Z)[kb]^T @ (D_u X)[kb, chunk]
            for s in range(NBSLOT if BC else 0):
                psum = ps_a.tile([128, 512], F32, tag="aps", name=f"bps{s}")
                for kg in range(KB // KGRP):
                    rt = gpb.tile([128, KGRP, 512], FP8, tag="gbr")
                    nc.scalar.dma_start(
                        rt[:], gb_r.ap()[s, kg].rearrange(
                            "p (k f) -> p k f", k=KGRP))
                    for kk in range(KGRP):
                        kb = kg * KGRP + kk
                        sz = zgp.tile([128, 128], FP8, tag="szb")
                        nc.vector.tensor_scalar_mul(sz[:], zgt[:, kb, :NZ],
                                                    combB[s][:, kb:kb + 1])
                        nc.tensor.matmul(psum[:], sz[:], rt[:, kk, :],
                                         start=(kb == 0), stop=(kb == KB - 1))
                nc.scalar.activation(trash[:], psum[:], AF.Square,
                                     accum_out=acc[:, 16 + s:17 + s])

            # C: psum [NZ, NZ] = sum_kb (D_v Z)[kb]^T @ (D_v Z)[kb]
            if BC:
                psum = ps_sm.tile([128, 128], F32, tag="sm", name="cps")
                for kb in range(KB):
                    sc = zgp.tile([128, 128], BF16, tag="szc")
                    nc.scalar.activation(sc[:], zgt[:, kb, :NZ], AF.Copy,
                                         scale=combC[:, kb:kb + 1])
                    nc.tensor.matmul(psum[:], sc[:], sc[:],
                                     start=(kb == 0), stop=(kb == KB - 1))
                nc.scalar.activation(trash[:, :128], psum[:], AF.Square,
                                     accum_out=acc[:, 18:19])

            # ---------------- loss partial ----------------
            ra = pp.tile([128, 1], F32, tag="ra")
            rb = pp.tile([128, 1], F32, tag="rb")
            nc.vector.tensor_reduce(ra[:], acc[:, 0:16], mybir.AxisListType.X,
                                    ALU.add)
            nc.vector.tensor_scalar_mul(ra[:], ra[:], float(FP8_SCALE ** -4))
            nc.vector.tensor_reduce(rb[:], acc[:, 16:18], mybir.AxisListType.X,
                                    ALU.add)
            nc.vector.tensor_scalar_mul(
                rb[:], rb[:], float(-2.0 * (FP8_SCALE * SZ_SCALE) ** -2))
            nc.vector.tensor_tensor(ra[:], ra[:], rb[:], ALU.add)
            nc.vector.tensor_tensor(ra[:], ra[:], acc[:, 18:19], ALU.add)
            lt = pp.tile([128, 1], F32, tag="lossf")
            nc.vector.tensor_copy(lt[:], ra[:])
            nc.sync.dma_start(lossp.ap()[:, :], lt[:])

    nc.compile()
    return nc


_NC = None


def _get_nc():
    global _NC
    if _NC is None:
        _NC = build_nc()
    return _NC


def _prep_in_maps(inputs):
    bf = ml_dtypes.bfloat16
    f8 = mybir.dt.np(FP8)
    xs = [np.ascontiguousarray(np.asarray(inputs[f"x{i}"], dtype=np.float32))
          for i in range(NV)]
    we = np.asarray(inputs["we"], dtype=np.float32)
    p = inputs["params"]
    enc = p["enc"]
    var = np.asarray(p["variables"], dtype=np.float32)

    we_s = we * var[None, :]
    nw = we_s / we_s.sum(axis=1, keepdims=True)
    u = []
    for i in range(NV):
        r = np.sqrt((xs[i] ** 2).sum(axis=1))
        u.append(we_s[:, i] / np.maximum(r, 1e-12))

    def swz(w, kp, mp):
        # -> [mp//128, 128, kp]: out[m, p, k*128+f] = w_pad[k*128+p, m*128+f]
        k, m = w.shape
        out = np.zeros((kp, mp), np.float32)
        out[:k, :m] = np.asarray(w, np.float32)
        out = out.reshape(kp // 128, 128, mp // 128, 128)
        out = out.transpose(2, 1, 0, 3).reshape(mp // 128, 128, kp)
        return np.ascontiguousarray(out).astype(bf)

    def padv(b, n):
        out = np.zeros((n,), np.float32)
        out[: b.shape[0]] = np.asarray(b, np.float32)
        return out

    shared = {}
    for v in range(NV):
        e = enc[v]
        shared[f"w1_{v}"] = swz(e["W1"], D[v], D0P[v])
        shared[f"w2_{v}"] = swz(e["W2"], D0P[v], HIDP)
        shared[f"wz_{v}"] = swz(e["Wz"], HIDP, 128)
    shared["regw"] = np.asarray(p["reg_W"], np.float32).astype(bf)

    CCOLS = sum(D0P[v] // 128 + HIDP // 128 + 3 for v in range(NV)) + 3 * KB + 2
    # bw/cw columns are per-core; consts base (weights/biases) is shared
    cbase = np.zeros((128, CCOLS), np.float32)
    off = 0
    for v in range(NV):
        e = enc[v]
        nb1 = D0P[v] // 128
        cbase[:, off:off + nb1] = padv(e["b1"], D0P[v]).reshape(nb1, 128).T
        off += nb1
        nb2 = HIDP // 128
        cbase[:, off:off + nb2] = padv(e["b2"], HIDP).reshape(nb2, 128).T
        off += nb2
        cbase[:, off] = padv(e["bz"], NZ)
        cbase[:, off + 1] = np.asarray(e["gamma"], np.float32)
        cbase[:, off + 2] = np.asarray(e["beta"], np.float32)
        off += 3
    bw_off = off
    cw_off = off + NBSLOT * KB
    regb_off = cw_off + KB
    cbase[:NL, regb_off] = np.asarray(p["reg_b"], np.float32)
    cbase[:, regb_off + 1] = BN_EPS

    sqrt2 = np.float32(math.sqrt(2.0))
    in_maps = []
    for c in range(NCORES):
        rows = slice(c * R, (c + 1) * R)
        m = dict(shared)
        for v in range(NV):
            kd = D[v] // 128
            xtv = xs[v][rows].T.reshape(kd, 128, R).transpose(1, 0, 2)
            m[f"xt{v}"] = np.ascontiguousarray(
                xtv.reshape(128, kd * R)).astype(bf)
        m["nwl"] = np.ascontiguousarray(nw[rows].T)

        gal = np.zeros((NASLOT, N, 512), f8)
        gar = np.zeros((NASLOT, N, 512), f8)
        KGG = KB // KGRP
        for s in range(NASLOT):
            idx = c * NASLOT + s
            if idx >= len(A_PIECES):
                continue
            v, cs, w, ms = A_PIECES[idx]
            uw = u[v][:, None] * np.float32(FP8_SCALE)
            fac = sqrt2 if ms < cs else np.float32(1.0)
            lw = min(512, D[v] - ms)
            gal[s, :, :lw] = (xs[v][:, ms:ms + lw] * (uw * fac)).astype(f8)
            gar[s, :, :w] = (xs[v][:, cs:cs + w] * uw).astype(f8)
        def gsw(g, nslot):
            # [S, N, 512] -> [S, KGG, 128, KGRP*512]
            g = g.reshape(nslot, KGG, KGRP, 128, 512).transpose(0, 1, 3, 2, 4)
            return np.ascontiguousarray(
                g.reshape(nslot, KGG, 128, KGRP * 512))
        m["ga_l"] = gsw(gal, NASLOT)
        m["ga_r"] = gsw(gar, NASLOT)

        gbr = np.zeros((NBSLOT, N, 512), f8)
        cc = cbase.copy()
        for s in range(NBSLOT):
            v, cs, w = B_UNITS[c * NBSLOT + s]
            gbr[s, :, :w] = (xs[v][:, cs:cs + w]
                             * (u[v][:, None] * np.float32(FP8_SCALE))).astype(f8)
            cc[:, bw_off + s * KB:bw_off + (s + 1) * KB] = \
                we_s[:, v].reshape(KB, 128).T * np.float32(SZ_SCALE)
        m["gb_r"] = gsw(gbr, NBSLOT)
        if c < NV:
            cc[:, cw_off:cw_off + KB] = we_s[:, c].reshape(KB, 128).T
        m["consts"] = cc
        in_maps.append(m)
    return in_maps


def kernel(**inputs):
    nc = _get_nc()
    in_maps = _prep_in_maps(inputs)
    res = run_bass_kernel_spmd(nc, in_maps, core_ids=list(range(NCORES)))
    r = res.results
    yL = np.concatenate([r[c]["yo"] for c in range(NCORES)], axis=0)
    z = np.concatenate([r[c]["zo"] for c in range(NCORES)], axis=0)
    zs = np.concatenate([r[c]["zso"] for c in range(NCORES)], axis=1)
    ysp = np.concatenate([r[c]["yspo"] for c in range(NCORES)], axis=1)
    total = np.float64(0.0)
    for c in range(NCORES):
        total += np.float64(r[c]["lossp"].sum())
    loss = np.float32(total / (float(N) * float(N)) / 6.0)
    return (yL, z, zs, ysp, loss)
